# revision 1
# baseline (speedup 1.0000x reference)
"""Trainium2 Bass kernel for nn_AttentionLayer (B=8,T=12,S=512,D=128,H=8).

Sharding: data-parallel over batch; core b handles query/key/value[b].
Host pre-transposes per-(t) input slabs to [D, S] so every on-chip matmul
consumes operands with the contraction dim on partitions (no on-chip
transposes).

PE constraint: matmul operand/output APs may only start at partition
0/32/64 (lhsT and rhs at the SAME base). Hence:
  - scores run per head PAIR at K=32, with the complementary 16 rows of
    the moving operand zeroed via a per-partition mask multiply
    (qT_even / qT_odd), split into a [0:96] tensor (pairs g=0,1,2) and a
    g3 tensor holding rows 96:128 shifted to base 0.
  - attnV outputs go 3 heads per PSUM bank at offsets {0,32,64}
    (banks A: heads 0-2, B: 3-5, C: 6-7), each head writing
    rows [32j, 32j+17): 16 attn rows + 1 sumexp row (ones column in
    the stationary operand).
  - softmax denominator: av banks are copied to SBUF; a selector matmul
    per bank gathers the sumexp rows and broadcasts them over the head's
    16 rows; reciprocal = exp(-ln(x)) on ACT (Ln/Exp share a table set;
    DVE reciprocal is 8 cyc/elem); one elementwise multiply per bank.
  - outproj accumulates 8 per-head K=16 matmuls (base 32j on both sides
    via host-aligned WoA/WoB/WoC) plus a K=1 bias-broadcast matmul with
    c = bv @ Wo + bo (bv folds through softmax: attn rows sum to 1).
"""

import sys

sys.path.insert(0, "/opt/trn_rl_repo")

import numpy as np

B, T, S, D = 8, 12, 512, 128
H, HD = 8, 16
NT = S // 128  # 4 s-tiles of 128
PATCH_ACT_TABLES = True
REPS = 1  # benchmarking: replicate the t-loop to measure steady-state slope

# head -> (bank, slot): banks A=0 (heads 0-2), B=1 (3-5), C=2 (6,7)
def head_bank(h):
    return (h // 3, h % 3) if h < 6 else (2, h - 6)


BANK_HEADS = [[0, 1, 2], [3, 4, 5], [6, 7]]
BANK_ROWS = [96, 96, 64]


def build_bass():
    from contextlib import ExitStack

    import concourse.bass as bass
    from concourse import bacc, mybir
    import concourse.tile as tile

    fp32 = mybir.dt.float32
    f32r = mybir.dt.float32r
    AF = mybir.ActivationFunctionType

    # Our only ACT functions are Exp and Ln; both live in the
    # natural_log_exp_and_others table set. Left to itself the table-load
    # pass maps Exp->exp_and_others and Ln->natural_log, paying a ~1.3us
    # table reload twice per timestep. Blank out the single-function sets
    # (keeping dict order, so act_func_set_id indices stay aligned with
    # act_info.json) to force the combined set: one load for the whole
    # kernel.
    import concourse.hw_specs as hw_specs
    from concourse import bacc as bacc_mod

    _orig_tables = hw_specs.get_activation_tables

    def _patched_tables(arch):
        t = dict(_orig_tables(arch))
        for k in ("exp_and_others", "natural_log"):
            if k in t:
                t[k] = set()
        return t

    nc = bacc.Bacc(None, target_bir_lowering=False)

    qT_d = nc.declare_dram_parameter("qT", [T, D, S], f32r, isOutput=False)
    kT_d = nc.declare_dram_parameter("kT", [T, D, S], f32r, isOutput=False)
    vT_d = nc.declare_dram_parameter("vT", [T, D, S], f32r, isOutput=False)
    Wq_d = nc.declare_dram_parameter("Wq", [D, D], f32r, isOutput=False)
    Wk_d = nc.declare_dram_parameter("Wk", [D, D], f32r, isOutput=False)
    Wv_d = nc.declare_dram_parameter("Wv", [D, D], f32r, isOutput=False)
    Wo_d = nc.declare_dram_parameter("Wox", [3, D, D], f32r, isOutput=False)
    bqe_d = nc.declare_dram_parameter("bqe", [D, 1], fp32, isOutput=False)
    bqo_d = nc.declare_dram_parameter("bqo", [D, 1], fp32, isOutput=False)
    bk_d = nc.declare_dram_parameter("bkr", [D, 1], fp32, isOutput=False)
    c_d = nc.declare_dram_parameter("cvec", [D, 1], fp32, isOutput=False)
    selg_d = nc.declare_dram_parameter("selg", [3, 96, 8], f32r, isOutput=False)
    sel8_d = nc.declare_dram_parameter("sel8", [3, 8, D], f32r, isOutput=False)
    maskE_d = nc.declare_dram_parameter("maskE", [D, 1], fp32, isOutput=False)
    maskO_d = nc.declare_dram_parameter("maskO", [D, 1], fp32, isOutput=False)
    # output is produced transposed ([d, s] per t); host untransposes
    out_d = nc.declare_dram_parameter("out", [T, D, S], fp32, isOutput=True)

    with ExitStack() as ctx:
        tc = ctx.enter_context(tile.TileContext(nc))
        consts = ctx.enter_context(tc.tile_pool(name="consts", bufs=1))
        io = ctx.enter_context(tc.tile_pool(name="io", bufs=3))
        proj = ctx.enter_context(tc.tile_pool(name="proj", bufs=2))
        expp = ctx.enter_context(tc.tile_pool(name="expp", bufs=3))
        normp = ctx.enter_context(tc.tile_pool(name="normp", bufs=2))
        outp = ctx.enter_context(tc.tile_pool(name="outp", bufs=4))
        ps = ctx.enter_context(
            tc.tile_pool(name="ps", bufs=2, space=bass.MemorySpace.PSUM)
        )
        psav = ctx.enter_context(
            tc.tile_pool(name="psav", bufs=2, space=bass.MemorySpace.PSUM)
        )
        pspj = ctx.enter_context(
            tc.tile_pool(name="pspj", bufs=2, space=bass.MemorySpace.PSUM)
        )

        # ---- constants (loaded once) ----
        Wq_sb = consts.tile([D, D], f32r)
        Wk_sb = consts.tile([D, D], f32r)
        Wv_sb = consts.tile([D, D], f32r)
        Wo_sb = consts.tile([D, 3, D], f32r)
        bqe_sb = consts.tile([D, 1], fp32)
        bqo_sb = consts.tile([D, 1], fp32)
        bk_sb = consts.tile([D, 1], fp32)
        c_sb = consts.tile([D, 1], fp32)
        selg_sb = consts.tile([96, 3, 8], f32r)
        sel8_sb = consts.tile([8, 3, D], f32r)
        maskE_sb = consts.tile([D, 1], fp32)
        maskO_sb = consts.tile([D, 1], fp32)
        nc.sync.dma_start(out=Wq_sb, in_=Wq_d[:])
        nc.sync.dma_start(out=Wk_sb, in_=Wk_d[:])
        nc.sync.dma_start(out=Wv_sb, in_=Wv_d[:])
        nc.gpsimd.dma_start(out=Wo_sb, in_=Wo_d.rearrange("x p f -> p x f"))
        nc.sync.dma_start(out=bqe_sb, in_=bqe_d[:])
        nc.sync.dma_start(out=bqo_sb, in_=bqo_d[:])
        nc.sync.dma_start(out=bk_sb, in_=bk_d[:])
        nc.sync.dma_start(out=c_sb, in_=c_d[:])
        nc.gpsimd.dma_start(out=selg_sb, in_=selg_d.rearrange("x p f -> p x f"))
        nc.gpsimd.dma_start(out=sel8_sb, in_=sel8_d.rearrange("x p f -> p x f"))
        nc.sync.dma_start(out=maskE_sb, in_=maskE_d[:])
        nc.sync.dma_start(out=maskO_sb, in_=maskO_d[:])

        a_sb_hist = {}

        def emit_outproj(tt):
            # transposed output projection: outT[dout, s] = sum_d Wo[d, dout]
            # * attnT[d, s], accumulated per head block; +c via per-partition
            # scalar add fused into the PSUM->SBUF copy.
            # full-K matmuls per bank: the gap rows are zero on BOTH sides
            # (host zeros Wox's non-head rows; av_sb gaps stay memset-zero),
            # so K=128 contractions give exactly the per-head sum. Crucially
            # all matmuls of this accumulation group share tile_position
            # (0,0) -- accumulating at mixed bases faults the hardware.
            a_prev = a_sb_hist.pop(tt)
            ps_ot = pspj.tile([D, S], fp32, tag="pspj", name=f"ps_ot{tt}")
            for x in range(3):
                nc.tensor.matmul(
                    ps_ot,
                    Wo_sb[:, x, :],
                    a_prev[:, x, :],
                    start=(x == 0),
                    stop=(x == 2),
                )
            o_sb = outp.tile([D, S], fp32, tag="o_sb", name=f"o_sb{tt}")
            nc.vector.tensor_scalar_add(o_sb, ps_ot, c_sb)
            nc.sync.dma_start(out=out_d[tt % T], in_=o_sb)

        norm_steps = {}

        # manual double-buffer for av_sb (persistent tiles so the gap rows
        # between head blocks stay defined after a single startup memset;
        # Tile's access tracking handles the t/t+2 reuse hazards)
        av_sb_pair = []
        for i in range(2):
            av_sbP = normp.tile([D, 3, S], f32r, tag=f"av_sbP{i}", name=f"av_sbP{i}")
            nc.vector.memset(av_sbP.bitcast(fp32), 0.0)
            av_sb_pair.append(av_sbP)

        def make_norm_steps(t, av_sb):
            # The four pieces of softmax normalization for timestep t; they
            # are emitted interleaved into timestep t+1's head loop so the
            # PE/ACT streams never stall on this chain.
            # 1) gather the 8 sumexp rows into one [8, 512] psum tile
            # 2) reciprocal on just those rows via exp(-ln(x)) on ACT
            # 3) broadcast reciprocals over each head's 16 rows (selector
            #    matmuls)
            # 4) elementwise multiply -> normalized attn (a_sb)
            state = {}

            def s0():
                ps_g = ps.tile([D, 2, S], fp32, tag="ps_s", name=f"ps_g{t}")
                for x in range(3):
                    nc.tensor.matmul(
                        ps_g[0:8, 0, :],
                        selg_sb[0:96, x, :],
                        av_sb[0:96, x, :],
                        start=(x == 0),
                        stop=(x == 2),
                    )
                state["ps_g"] = ps_g

            def s1():
                lnS = normp.tile([8, S], fp32, tag="lnS", name=f"lnS{t}")
                nc.scalar.activation(lnS, state["ps_g"][0:8, 0, :], AF.Ln)
                recip_sb = normp.tile([8, S], f32r, tag="recip", name=f"recip{t}")
                nc.scalar.activation(recip_sb, lnS, AF.Exp, scale=-1.0)
                state["recip"] = recip_sb

            def s2():
                ps_R1 = ps.tile([D, 2, S], fp32, tag="ps_s", name=f"ps_R1{t}")
                for x in range(2):
                    nc.tensor.matmul(
                        ps_R1[:, x, :],
                        sel8_sb[:, x, :],
                        state["recip"],
                        start=True,
                        stop=True,
                    )
                ps_R2 = ps.tile([D, 2, S], fp32, tag="ps_s", name=f"ps_R2{t}")
                nc.tensor.matmul(
                    ps_R2[:, 0, :],
                    sel8_sb[:, 2, :],
                    state["recip"],
                    start=True,
                    stop=True,
                )
                state["ps_R1"] = ps_R1
                state["ps_R2"] = ps_R2

            def s3():
                a_sb = normp.tile([D, 3, S], f32r, tag="a_sb", name=f"a_sb{t}")
                nc.vector.tensor_mul(
                    a_sb[:, 0:2, :], av_sb[:, 0:2, :], state["ps_R1"]
                )
                nc.vector.tensor_mul(
                    a_sb[:, 2, :], av_sb[:, 2, :], state["ps_R2"][:, 0, :]
                )
                a_sb_hist[t] = a_sb

            return [s0, s1, s2, s3]

        for rt in range(REPS * T):
            t = rt % T
            # ---- load inputs (already [D, S] transposed on host) ----
            qT_in = io.tile([D, S], f32r, tag="qT_in")
            kT_in = io.tile([D, S], f32r, tag="kT_in")
            vT_in = io.tile([D, S], f32r, tag="vT_in")
            nc.sync.dma_start(out=qT_in, in_=qT_d[t])
            nc.sync.dma_start(out=kT_in, in_=kT_d[t])
            nc.sync.dma_start(out=vT_in, in_=vT_d[t])

            # ---- projections ----
            # masked parity copies (zero the other head of each 32-row pair)
            # with the bias fused: out = psum * mask + bias*mask.
            ps_qT = pspj.tile([D, S], fp32, tag="pspj")
            nc.tensor.matmul(ps_qT, Wq_sb, qT_in, start=True, stop=True)
            qT_ev = proj.tile([96, S], f32r, tag="qT_ev")
            qT_od = proj.tile([96, S], f32r, tag="qT_od")
            qT_ev3 = proj.tile([32, S], f32r, tag="qT_ev3")
            qT_od3 = proj.tile([32, S], f32r, tag="qT_od3")
            mlt, add = mybir.AluOpType.mult, mybir.AluOpType.add
            nc.vector.tensor_scalar(
                qT_ev, ps_qT[0:96, :], maskE_sb[0:96, :], bqe_sb[0:96, :], mlt, add
            )
            nc.vector.tensor_scalar(
                qT_od, ps_qT[0:96, :], maskO_sb[0:96, :], bqo_sb[0:96, :], mlt, add
            )
            nc.vector.tensor_scalar(
                qT_ev3, ps_qT[96:128, :], maskE_sb[96:128, :], bqe_sb[96:128, :],
                mlt, add,
            )
            nc.vector.tensor_scalar(
                qT_od3, ps_qT[96:128, :], maskO_sb[96:128, :], bqo_sb[96:128, :],
                mlt, add,
            )

            ps_kT = pspj.tile([D, S], fp32, tag="pspj")
            nc.tensor.matmul(ps_kT, Wk_sb, kT_in, start=True, stop=True)
            kT_sb = proj.tile([96, S], f32r, tag="kT_sb")
            kT_g3 = proj.tile([32, S], f32r, tag="kT_g3")
            nc.vector.tensor_scalar_add(kT_sb, ps_kT[0:96, :], bk_sb[0:96, :])
            nc.vector.tensor_scalar_add(kT_g3, ps_kT[96:128, :], bk_sb[96:128, :])

            # v in natural [s, d] layout, interleaved with ones columns:
            # v_sb[:, st, h, 0:16] = v[st*128+p, 16h+j], v_sb[:, st, h, 16] = 1
            ps_v = pspj.tile([D, S], fp32, tag="pspj")
            for st in range(NT):
                nc.tensor.matmul(
                    ps_v[:, st * 128 : (st + 1) * 128],
                    vT_in[:, st * 128 : (st + 1) * 128],
                    Wv_sb,
                    start=True,
                    stop=True,
                )
            v_sb = proj.tile([D, NT, H, HD + 1], f32r, tag="v_sb")
            nc.vector.memset(v_sb[:, :, :, HD : HD + 1].bitcast(fp32), 1.0)
            nc.vector.tensor_copy(
                v_sb[:, :, :, 0:HD],
                ps_v.rearrange("p (st h j) -> p st h j", st=NT, h=H),
            )

            # ---- attention ----
            # per-head PSUM accumulator tile (bufs=2), copied into av_sb
            # right after each head completes; the previous timestep's
            # normalize steps are interleaved at head boundaries.
            av_sb = av_sb_pair[rt % 2]
            for h in range(H):
                g = h // 2
                if g < 3:
                    kT_l, qe, qo = kT_sb, qT_ev, qT_od
                    base = 32 * g
                else:
                    kT_l, qe, qo = kT_g3, qT_ev3, qT_od3
                    base = 0
                qT_par = qe if h % 2 == 0 else qo
                bank, j = head_bank(h)
                expT = expp.tile([D, NT, S], f32r, tag="expT")
                for stp in range(NT // 2):
                    ps_s = ps.tile([D, 2, S], fp32, tag="ps_s")
                    for k in range(2):
                        st = 2 * stp + k
                        nc.tensor.matmul(
                            ps_s[:, k, :],
                            kT_l[base : base + 32, st * 128 : (st + 1) * 128],
                            qT_par[base : base + 32, :],
                            start=True,
                            stop=True,
                        )
                    nc.scalar.activation(
                        expT[:, 2 * stp : 2 * stp + 2, :], ps_s, AF.Exp, scale=0.25
                    )
                av_h = psav.tile([17, S], fp32, tag="av_h")
                for st in range(NT):
                    nc.tensor.matmul(
                        av_h,
                        v_sb[:, st, h, :],
                        expT[:, st, :],
                        start=(st == 0),
                        stop=(st == NT - 1),
                    )
                nc.vector.tensor_copy(av_sb[32 * j : 32 * j + 17, bank, :], av_h)
                if rt > 0 and h < 4:
                    norm_steps[rt - 1][h]()

            # software-pipelined output projection for t-1: its inputs
            # (a_sb of t-1) finished during this t's head loop, so PE
            # never stalls on the normalize chain.
            if rt > 0:
                emit_outproj(rt - 1)

            norm_steps[rt] = make_norm_steps(rt, av_sb)

        for step in norm_steps[REPS * T - 1]:
            step()
        emit_outproj(REPS * T - 1)

    if PATCH_ACT_TABLES:
        bacc_mod.get_activation_tables = _patched_tables
    try:
        nc.compile()
    finally:
        bacc_mod.get_activation_tables = _orig_tables
    return nc


def make_in_maps(query, key, value, Wq, bq, Wk, bk, Wv, bv, Wo, bo):
    f = np.float32
    Wo = np.asarray(Wo, f)
    c = (np.asarray(bv, f) @ Wo + np.asarray(bo, f)).reshape(D, 1)
    selg = np.zeros((3, 96, 8), f)
    sel8 = np.zeros((3, 8, D), f)
    Wox = np.zeros((3, D, D), f)
    for h in range(H):
        bank, j = head_bank(h)
        selg[bank, 32 * j + 16, h] = 1.0
        sel8[bank, h, 32 * j : 32 * j + 32] = 1.0
        Wox[bank, 32 * j : 32 * j + 16, :] = Wo[HD * h : HD * (h + 1), :]
    maskE = np.zeros((D, 1), f)
    maskO = np.zeros((D, 1), f)
    for p in range(D):
        if (p % 32) < 16:
            maskE[p] = 1.0
        else:
            maskO[p] = 1.0
    bq = np.ascontiguousarray(bq, f).reshape(D, 1)
    shared = {
        "Wq": np.ascontiguousarray(Wq, f),
        "Wk": np.ascontiguousarray(Wk, f),
        "Wv": np.ascontiguousarray(Wv, f),
        "Wox": Wox,
        "bqe": bq * maskE,
        "bqo": bq * maskO,
        "bkr": np.ascontiguousarray(bk, f).reshape(D, 1),
        "cvec": c,
        "selg": selg,
        "sel8": sel8,
        "maskE": maskE,
        "maskO": maskO,
    }
    in_maps = []
    for b in range(B):
        m = dict(shared)
        m["qT"] = np.ascontiguousarray(np.asarray(query[b], f).transpose(0, 2, 1))
        m["kT"] = np.ascontiguousarray(np.asarray(key[b], f).transpose(0, 2, 1))
        m["vT"] = np.ascontiguousarray(np.asarray(value[b], f).transpose(0, 2, 1))
        in_maps.append(m)
    return in_maps


def kernel(query, key, value, Wq, bq, Wk, bk, Wv, bv, Wo, bo):
    from concourse.bass_utils import run_bass_kernel_spmd

    nc = build_bass()
    in_maps = make_in_maps(query, key, value, Wq, bq, Wk, bk, Wv, bv, Wo, bo)
    res = run_bass_kernel_spmd(nc, in_maps, core_ids=list(range(B)))
    # device output is [T, D, S]; untranspose to [T, S, D]
    out = np.stack(
        [res.results[i]["out"].transpose(0, 2, 1) for i in range(B)]
    )
    return out



# revision 2
# speedup vs baseline: 1.1151x; 1.1151x over previous
"""Trainium2 Bass kernel for nn_AttentionLayer (B=8,T=12,S=512,D=128,H=8).

Sharding: data-parallel over batch; core b handles query/key/value[b].
Host pre-transposes per-(t) input slabs to [D, S] so every on-chip matmul
consumes operands with the contraction dim on partitions (no on-chip
transposes).

All matmul operand paths are bf16 (real-HW f32r matmuls run in fp32_mode
at ~2 cycles/row; bf16 runs at 1 cycle/row). PSUM accumulation stays
fp32; softmax Ln stays fp32 (abs error there is exponentiated).

PE constraint: matmul operand/output APs may only start at partition
0/32/64 (lhsT and rhs at the SAME base). Hence:
  - scores run per head PAIR at K=32, with the complementary 16 rows of
    the moving operand zeroed via a per-partition mask multiply
    (qT_even / qT_odd), split into a [0:96] tensor (pairs g=0,1,2) and a
    g3 tensor holding rows 96:128 shifted to base 0.
  - attnV outputs go 3 heads per PSUM bank at offsets {0,32,64}
    (banks A: heads 0-2, B: 3-5, C: 6-7), each head writing
    rows [32j, 32j+17): 16 attn rows + 1 sumexp row (ones column in
    the stationary operand).
  - softmax denominator: av banks are copied to SBUF; a selector matmul
    per bank gathers the sumexp rows and broadcasts them over the head's
    16 rows; reciprocal = exp(-ln(x)) on ACT (Ln/Exp share a table set;
    DVE reciprocal is 8 cyc/elem); one elementwise multiply per bank.
  - outproj accumulates per-bank K=128 matmuls (host zeros Wox gap rows,
    av_sb gaps stay memset-zero) plus a bias add with
    c = bv @ Wo + bo (bv folds through softmax: attn rows sum to 1).
"""

import sys

sys.path.insert(0, "/opt/trn_rl_repo")

import numpy as np
import ml_dtypes

BF16 = ml_dtypes.bfloat16

B, T, S, D = 8, 12, 512, 128
H, HD = 8, 16
NT = S // 128  # 4 s-tiles of 128
PATCH_ACT_TABLES = True
REPS = 1  # benchmarking: replicate the t-loop to measure steady-state slope

# head -> (bank, slot): banks A=0 (heads 0-2), B=1 (3-5), C=2 (6,7)
def head_bank(h):
    return (h // 3, h % 3) if h < 6 else (2, h - 6)


BANK_HEADS = [[0, 1, 2], [3, 4, 5], [6, 7]]
BANK_ROWS = [96, 96, 64]


def build_bass():
    from contextlib import ExitStack

    import concourse.bass as bass
    from concourse import bacc, mybir
    import concourse.tile as tile

    fp32 = mybir.dt.float32
    bf16 = mybir.dt.bfloat16
    AF = mybir.ActivationFunctionType

    # Our only ACT functions are Exp and Ln; both live in the
    # natural_log_exp_and_others table set. Left to itself the table-load
    # pass maps Exp->exp_and_others and Ln->natural_log, paying a ~1.3us
    # table reload twice per timestep. Blank out the single-function sets
    # (keeping dict order, so act_func_set_id indices stay aligned with
    # act_info.json) to force the combined set: one load for the whole
    # kernel.
    import concourse.hw_specs as hw_specs
    from concourse import bacc as bacc_mod

    _orig_tables = hw_specs.get_activation_tables

    def _patched_tables(arch):
        t = dict(_orig_tables(arch))
        for k in ("exp_and_others", "natural_log"):
            if k in t:
                t[k] = set()
        return t

    nc = bacc.Bacc(None, target_bir_lowering=False)

    qT_d = nc.declare_dram_parameter("qT", [T, D, S], bf16, isOutput=False)
    kT_d = nc.declare_dram_parameter("kT", [T, D, S], bf16, isOutput=False)
    vT_d = nc.declare_dram_parameter("vT", [T, D, S], bf16, isOutput=False)
    Wq_d = nc.declare_dram_parameter("Wq", [D, D], bf16, isOutput=False)
    Wk_d = nc.declare_dram_parameter("Wk", [D, D], bf16, isOutput=False)
    Wv_d = nc.declare_dram_parameter("Wv", [D, D], bf16, isOutput=False)
    Wo_d = nc.declare_dram_parameter("Wox", [3, D, D], bf16, isOutput=False)
    bqe_d = nc.declare_dram_parameter("bqe", [D, 1], fp32, isOutput=False)
    bqo_d = nc.declare_dram_parameter("bqo", [D, 1], fp32, isOutput=False)
    bk_d = nc.declare_dram_parameter("bkr", [D, 1], fp32, isOutput=False)
    c_d = nc.declare_dram_parameter("cvec", [D, 1], fp32, isOutput=False)
    selg_d = nc.declare_dram_parameter("selg", [3, 96, 8], bf16, isOutput=False)
    sel8_d = nc.declare_dram_parameter("sel8", [3, 8, D], bf16, isOutput=False)
    maskE_d = nc.declare_dram_parameter("maskE", [D, 1], fp32, isOutput=False)
    maskO_d = nc.declare_dram_parameter("maskO", [D, 1], fp32, isOutput=False)
    # output is produced transposed ([d, s] per t); host untransposes
    out_d = nc.declare_dram_parameter("out", [T, D, S], fp32, isOutput=True)

    with ExitStack() as ctx:
        tc = ctx.enter_context(tile.TileContext(nc))
        consts = ctx.enter_context(tc.tile_pool(name="consts", bufs=1))
        io = ctx.enter_context(tc.tile_pool(name="io", bufs=3))
        proj = ctx.enter_context(tc.tile_pool(name="proj", bufs=2))
        expp = ctx.enter_context(tc.tile_pool(name="expp", bufs=3))
        normp = ctx.enter_context(tc.tile_pool(name="normp", bufs=2))
        outp = ctx.enter_context(tc.tile_pool(name="outp", bufs=4))
        ps = ctx.enter_context(
            tc.tile_pool(name="ps", bufs=2, space=bass.MemorySpace.PSUM)
        )
        psav = ctx.enter_context(
            tc.tile_pool(name="psav", bufs=2, space=bass.MemorySpace.PSUM)
        )
        pspj = ctx.enter_context(
            tc.tile_pool(name="pspj", bufs=2, space=bass.MemorySpace.PSUM)
        )

        # ---- constants (loaded once) ----
        Wq_sb = consts.tile([D, D], bf16)
        Wk_sb = consts.tile([D, D], bf16)
        Wv_sb = consts.tile([D, D], bf16)
        Wo_sb = consts.tile([D, 3, D], bf16)
        bqe_sb = consts.tile([D, 1], fp32)
        bqo_sb = consts.tile([D, 1], fp32)
        bk_sb = consts.tile([D, 1], fp32)
        c_sb = consts.tile([D, 1], fp32)
        selg_sb = consts.tile([96, 3, 8], bf16)
        sel8_sb = consts.tile([8, 3, D], bf16)
        maskE_sb = consts.tile([D, 1], fp32)
        maskO_sb = consts.tile([D, 1], fp32)
        nc.sync.dma_start(out=Wq_sb, in_=Wq_d[:])
        nc.sync.dma_start(out=Wk_sb, in_=Wk_d[:])
        nc.sync.dma_start(out=Wv_sb, in_=Wv_d[:])
        nc.gpsimd.dma_start(out=Wo_sb, in_=Wo_d.rearrange("x p f -> p x f"))
        nc.sync.dma_start(out=bqe_sb, in_=bqe_d[:])
        nc.sync.dma_start(out=bqo_sb, in_=bqo_d[:])
        nc.sync.dma_start(out=bk_sb, in_=bk_d[:])
        nc.sync.dma_start(out=c_sb, in_=c_d[:])
        nc.gpsimd.dma_start(out=selg_sb, in_=selg_d.rearrange("x p f -> p x f"))
        nc.gpsimd.dma_start(out=sel8_sb, in_=sel8_d.rearrange("x p f -> p x f"))
        nc.sync.dma_start(out=maskE_sb, in_=maskE_d[:])
        nc.sync.dma_start(out=maskO_sb, in_=maskO_d[:])

        a_sb_hist = {}

        def emit_outproj(tt):
            # transposed output projection: outT[dout, s] = sum_d Wo[d, dout]
            # * attnT[d, s], accumulated per head block; +c via per-partition
            # scalar add fused into the PSUM->SBUF copy.
            # full-K matmuls per bank: the gap rows are zero on BOTH sides
            # (host zeros Wox's non-head rows; av_sb gaps stay memset-zero),
            # so K=128 contractions give exactly the per-head sum. Crucially
            # all matmuls of this accumulation group share tile_position
            # (0,0) -- accumulating at mixed bases faults the hardware.
            a_prev = a_sb_hist.pop(tt)
            ps_ot = pspj.tile([D, S], fp32, tag="pspj", name=f"ps_ot{tt}")
            for x in range(3):
                nc.tensor.matmul(
                    ps_ot,
                    Wo_sb[:, x, :],
                    a_prev[:, x, :],
                    start=(x == 0),
                    stop=(x == 2),
                )
            o_sb = outp.tile([D, S], fp32, tag="o_sb", name=f"o_sb{tt}")
            nc.vector.tensor_scalar_add(o_sb, ps_ot, c_sb)
            nc.sync.dma_start(out=out_d[tt % T], in_=o_sb)

        norm_steps = {}

        # manual double-buffer for av_sb (persistent tiles so the gap rows
        # between head blocks stay defined after a single startup memset;
        # Tile's access tracking handles the t/t+2 reuse hazards)
        av_sb_pair = []
        for i in range(2):
            av_sbP = normp.tile([D, 3, S], bf16, tag=f"av_sbP{i}", name=f"av_sbP{i}")
            nc.vector.memset(av_sbP, 0.0)
            av_sb_pair.append(av_sbP)

        def make_norm_steps(t, av_sb):
            # The four pieces of softmax normalization for timestep t; they
            # are emitted interleaved into timestep t+1's head loop so the
            # PE/ACT streams never stall on this chain.
            # 1) gather the 8 sumexp rows into one [8, 512] psum tile
            # 2) reciprocal on just those rows via exp(-ln(x)) on ACT
            # 3) broadcast reciprocals over each head's 16 rows (selector
            #    matmuls)
            # 4) elementwise multiply -> normalized attn (a_sb)
            state = {}

            def s0():
                ps_g = ps.tile([D, 2, S], fp32, tag="ps_s", name=f"ps_g{t}")
                for x in range(3):
                    nc.tensor.matmul(
                        ps_g[0:8, 0, :],
                        selg_sb[0:96, x, :],
                        av_sb[0:96, x, :],
                        start=(x == 0),
                        stop=(x == 2),
                    )
                state["ps_g"] = ps_g

            def s1():
                lnS = normp.tile([8, S], fp32, tag="lnS", name=f"lnS{t}")
                nc.scalar.activation(lnS, state["ps_g"][0:8, 0, :], AF.Ln)
                recip_sb = normp.tile([8, S], bf16, tag="recip", name=f"recip{t}")
                nc.scalar.activation(recip_sb, lnS, AF.Exp, scale=-1.0)
                state["recip"] = recip_sb

            def s2():
                ps_R1 = ps.tile([D, 2, S], fp32, tag="ps_s", name=f"ps_R1{t}")
                for x in range(2):
                    nc.tensor.matmul(
                        ps_R1[:, x, :],
                        sel8_sb[:, x, :],
                        state["recip"],
                        start=True,
                        stop=True,
                    )
                ps_R2 = ps.tile([D, 2, S], fp32, tag="ps_s", name=f"ps_R2{t}")
                nc.tensor.matmul(
                    ps_R2[:, 0, :],
                    sel8_sb[:, 2, :],
                    state["recip"],
                    start=True,
                    stop=True,
                )
                state["ps_R1"] = ps_R1
                state["ps_R2"] = ps_R2

            def s3():
                a_sb = normp.tile([D, 3, S], bf16, tag="a_sb", name=f"a_sb{t}")
                nc.vector.tensor_mul(
                    a_sb[:, 0:2, :], av_sb[:, 0:2, :], state["ps_R1"]
                )
                nc.vector.tensor_mul(
                    a_sb[:, 2, :], av_sb[:, 2, :], state["ps_R2"][:, 0, :]
                )
                a_sb_hist[t] = a_sb

            return [s0, s1, s2, s3]

        for rt in range(REPS * T):
            t = rt % T
            # ---- load inputs (already [D, S] transposed + bf16 on host) ----
            qT_in = io.tile([D, S], bf16, tag="qT_in")
            kT_in = io.tile([D, S], bf16, tag="kT_in")
            vT_in = io.tile([D, S], bf16, tag="vT_in")
            nc.sync.dma_start(out=qT_in, in_=qT_d[t])
            nc.sync.dma_start(out=kT_in, in_=kT_d[t])
            nc.sync.dma_start(out=vT_in, in_=vT_d[t])

            # ---- projections ----
            # masked parity copies (zero the other head of each 32-row pair)
            # with the bias fused: out = psum * mask + bias*mask.
            ps_qT = pspj.tile([D, S], fp32, tag="pspj")
            nc.tensor.matmul(ps_qT, Wq_sb, qT_in, start=True, stop=True)
            qT_ev = proj.tile([96, S], bf16, tag="qT_ev")
            qT_od = proj.tile([96, S], bf16, tag="qT_od")
            qT_ev3 = proj.tile([32, S], bf16, tag="qT_ev3")
            qT_od3 = proj.tile([32, S], bf16, tag="qT_od3")
            mlt, add = mybir.AluOpType.mult, mybir.AluOpType.add
            nc.vector.tensor_scalar(
                qT_ev, ps_qT[0:96, :], maskE_sb[0:96, :], bqe_sb[0:96, :], mlt, add
            )
            nc.vector.tensor_scalar(
                qT_od, ps_qT[0:96, :], maskO_sb[0:96, :], bqo_sb[0:96, :], mlt, add
            )
            nc.vector.tensor_scalar(
                qT_ev3, ps_qT[96:128, :], maskE_sb[96:128, :], bqe_sb[96:128, :],
                mlt, add,
            )
            nc.vector.tensor_scalar(
                qT_od3, ps_qT[96:128, :], maskO_sb[96:128, :], bqo_sb[96:128, :],
                mlt, add,
            )

            ps_kT = pspj.tile([D, S], fp32, tag="pspj")
            nc.tensor.matmul(ps_kT, Wk_sb, kT_in, start=True, stop=True)
            kT_sb = proj.tile([96, S], bf16, tag="kT_sb")
            kT_g3 = proj.tile([32, S], bf16, tag="kT_g3")
            nc.vector.tensor_scalar_add(kT_sb, ps_kT[0:96, :], bk_sb[0:96, :])
            nc.vector.tensor_scalar_add(kT_g3, ps_kT[96:128, :], bk_sb[96:128, :])

            # v in natural [s, d] layout, interleaved with ones columns:
            # v_sb[:, st, h, 0:16] = v[st*128+p, 16h+j], v_sb[:, st, h, 16] = 1
            ps_v = pspj.tile([D, S], fp32, tag="pspj")
            for st in range(NT):
                nc.tensor.matmul(
                    ps_v[:, st * 128 : (st + 1) * 128],
                    vT_in[:, st * 128 : (st + 1) * 128],
                    Wv_sb,
                    start=True,
                    stop=True,
                )
            v_sb = proj.tile([D, NT, H, HD + 1], bf16, tag="v_sb")
            nc.vector.memset(v_sb[:, :, :, HD : HD + 1], 1.0)
            nc.vector.tensor_copy(
                v_sb[:, :, :, 0:HD],
                ps_v.rearrange("p (st h j) -> p st h j", st=NT, h=H),
            )

            # ---- attention ----
            # per-head PSUM accumulator tile (bufs=2), copied into av_sb
            # right after each head completes; the previous timestep's
            # normalize steps are interleaved at head boundaries.
            av_sb = av_sb_pair[rt % 2]
            for h in range(H):
                g = h // 2
                if g < 3:
                    kT_l, qe, qo = kT_sb, qT_ev, qT_od
                    base = 32 * g
                else:
                    kT_l, qe, qo = kT_g3, qT_ev3, qT_od3
                    base = 0
                qT_par = qe if h % 2 == 0 else qo
                bank, j = head_bank(h)
                expT = expp.tile([D, NT, S], bf16, tag="expT")
                for stp in range(NT // 2):
                    ps_s = ps.tile([D, 2, S], fp32, tag="ps_s")
                    for k in range(2):
                        st = 2 * stp + k
                        nc.tensor.matmul(
                            ps_s[:, k, :],
                            kT_l[base : base + 32, st * 128 : (st + 1) * 128],
                            qT_par[base : base + 32, :],
                            start=True,
                            stop=True,
                        )
                    nc.scalar.activation(
                        expT[:, 2 * stp : 2 * stp + 2, :], ps_s, AF.Exp, scale=0.25
                    )
                av_h = psav.tile([17, S], fp32, tag="av_h")
                for st in range(NT):
                    nc.tensor.matmul(
                        av_h,
                        v_sb[:, st, h, :],
                        expT[:, st, :],
                        start=(st == 0),
                        stop=(st == NT - 1),
                    )
                nc.vector.tensor_copy(av_sb[32 * j : 32 * j + 17, bank, :], av_h)
                if rt > 0 and h < 4:
                    norm_steps[rt - 1][h]()

            # software-pipelined output projection for t-1: its inputs
            # (a_sb of t-1) finished during this t's head loop, so PE
            # never stalls on the normalize chain.
            if rt > 0:
                emit_outproj(rt - 1)

            norm_steps[rt] = make_norm_steps(rt, av_sb)

        for step in norm_steps[REPS * T - 1]:
            step()
        emit_outproj(REPS * T - 1)

    if PATCH_ACT_TABLES:
        bacc_mod.get_activation_tables = _patched_tables
    try:
        nc.compile()
    finally:
        bacc_mod.get_activation_tables = _orig_tables
    return nc


def make_in_maps(query, key, value, Wq, bq, Wk, bk, Wv, bv, Wo, bo):
    f = np.float32
    Wo = np.asarray(Wo, f)
    c = (np.asarray(bv, f) @ Wo + np.asarray(bo, f)).reshape(D, 1)
    selg = np.zeros((3, 96, 8), f)
    sel8 = np.zeros((3, 8, D), f)
    Wox = np.zeros((3, D, D), f)
    for h in range(H):
        bank, j = head_bank(h)
        selg[bank, 32 * j + 16, h] = 1.0
        sel8[bank, h, 32 * j : 32 * j + 32] = 1.0
        Wox[bank, 32 * j : 32 * j + 16, :] = Wo[HD * h : HD * (h + 1), :]
    maskE = np.zeros((D, 1), f)
    maskO = np.zeros((D, 1), f)
    for p in range(D):
        if (p % 32) < 16:
            maskE[p] = 1.0
        else:
            maskO[p] = 1.0
    bq = np.ascontiguousarray(bq, f).reshape(D, 1)
    shared = {
        "Wq": np.ascontiguousarray(Wq, f).astype(BF16),
        "Wk": np.ascontiguousarray(Wk, f).astype(BF16),
        "Wv": np.ascontiguousarray(Wv, f).astype(BF16),
        "Wox": Wox.astype(BF16),
        "bqe": bq * maskE,
        "bqo": bq * maskO,
        "bkr": np.ascontiguousarray(bk, f).reshape(D, 1),
        "cvec": c,
        "selg": selg.astype(BF16),
        "sel8": sel8.astype(BF16),
        "maskE": maskE,
        "maskO": maskO,
    }
    in_maps = []
    for b in range(B):
        m = dict(shared)
        m["qT"] = np.ascontiguousarray(
            np.asarray(query[b], f).transpose(0, 2, 1)
        ).astype(BF16)
        m["kT"] = np.ascontiguousarray(
            np.asarray(key[b], f).transpose(0, 2, 1)
        ).astype(BF16)
        m["vT"] = np.ascontiguousarray(
            np.asarray(value[b], f).transpose(0, 2, 1)
        ).astype(BF16)
        in_maps.append(m)
    return in_maps


def kernel(query, key, value, Wq, bq, Wk, bk, Wv, bv, Wo, bo):
    from concourse.bass_utils import run_bass_kernel_spmd

    nc = build_bass()
    in_maps = make_in_maps(query, key, value, Wq, bq, Wk, bk, Wv, bv, Wo, bo)
    res = run_bass_kernel_spmd(nc, in_maps, core_ids=list(range(B)))
    # device output is [T, D, S]; untranspose to [T, S, D]
    out = np.stack(
        [res.results[i]["out"].transpose(0, 2, 1) for i in range(B)]
    )
    return out


# revision 17
# speedup vs baseline: 1.1319x; 1.0150x over previous
"""Trainium2 Bass kernel for nn_AttentionLayer (B=8,T=12,S=512,D=128,H=8).

Sharding: data-parallel over batch; core b handles query/key/value[b].
Host pre-transposes per-(t) input slabs to [D, S] so every on-chip matmul
consumes operands with the contraction dim on partitions (no on-chip
transposes).

All matmul operand paths are bf16 (real-HW f32r matmuls run in fp32_mode
at ~2 cycles/row; bf16 runs at 1 cycle/row). PSUM accumulation stays
fp32; softmax Ln stays fp32 (abs error there is exponentiated).

PE constraint: matmul operand/output APs may only start at partition
0/32/64 (lhsT and rhs at the SAME base). Hence:
  - scores run per head PAIR at K=32, with the complementary 16 rows of
    the moving operand zeroed via a per-partition mask multiply
    (qT_even / qT_odd), split into a [0:96] tensor (pairs g=0,1,2) and a
    g3 tensor holding rows 96:128 shifted to base 0.
  - attnV outputs go 3 heads per PSUM bank at offsets {0,32,64}
    (banks A: heads 0-2, B: 3-5, C: 6-7), each head writing
    rows [32j, 32j+17): 16 attn rows + 1 sumexp row (ones column in
    the stationary operand).
  - softmax denominator: av banks are copied to SBUF; a selector matmul
    per bank gathers the sumexp rows and broadcasts them over the head's
    16 rows; reciprocal = exp(-ln(x)) on ACT (Ln/Exp share a table set;
    DVE reciprocal is 8 cyc/elem); one elementwise multiply per bank.
  - outproj accumulates per-bank K=128 matmuls (host zeros Wox gap rows,
    av_sb gaps stay memset-zero) plus a bias add with
    c = bv @ Wo + bo (bv folds through softmax: attn rows sum to 1).
"""

import sys

sys.path.insert(0, "/opt/trn_rl_repo")

import numpy as np
import ml_dtypes

BF16 = ml_dtypes.bfloat16

B, T, S, D = 8, 12, 512, 128
H, HD = 8, 16
NT = S // 128  # 4 s-tiles of 128
PATCH_ACT_TABLES = True
REPS = 1  # benchmarking: replicate the t-loop to measure steady-state slope

# head -> (bank, slot): banks A=0 (heads 0-2), B=1 (3-5), C=2 (6,7)
def head_bank(h):
    return (h // 3, h % 3) if h < 6 else (2, h - 6)


BANK_HEADS = [[0, 1, 2], [3, 4, 5], [6, 7]]
BANK_ROWS = [96, 96, 64]


def build_bass():
    from contextlib import ExitStack

    import concourse.bass as bass
    from concourse import bacc, mybir
    import concourse.tile as tile

    fp32 = mybir.dt.float32
    bf16 = mybir.dt.bfloat16
    AF = mybir.ActivationFunctionType

    # Our only ACT functions are Exp and Ln; both live in the
    # natural_log_exp_and_others table set. Left to itself the table-load
    # pass maps Exp->exp_and_others and Ln->natural_log, paying a ~1.3us
    # table reload twice per timestep. Blank out the single-function sets
    # (keeping dict order, so act_func_set_id indices stay aligned with
    # act_info.json) to force the combined set: one load for the whole
    # kernel.
    import concourse.hw_specs as hw_specs
    from concourse import bacc as bacc_mod

    _orig_tables = hw_specs.get_activation_tables

    def _patched_tables(arch):
        t = dict(_orig_tables(arch))
        for k in ("exp_and_others", "natural_log"):
            if k in t:
                t[k] = set()
        return t

    nc = bacc.Bacc(None, target_bir_lowering=False)

    qT_d = nc.declare_dram_parameter("qT", [T, D, S], bf16, isOutput=False)
    kT_d = nc.declare_dram_parameter("kT", [T, D, S], bf16, isOutput=False)
    vT_d = nc.declare_dram_parameter("vT", [T, D, S], bf16, isOutput=False)
    Wq_d = nc.declare_dram_parameter("Wq", [D, D], bf16, isOutput=False)
    Wk_d = nc.declare_dram_parameter("Wk", [D, D], bf16, isOutput=False)
    Wv_d = nc.declare_dram_parameter("Wv", [D, D], bf16, isOutput=False)
    Wo_d = nc.declare_dram_parameter("Wox", [3, D, D], bf16, isOutput=False)
    bqe_d = nc.declare_dram_parameter("bqe", [D, 1], fp32, isOutput=False)
    bqo_d = nc.declare_dram_parameter("bqo", [D, 1], fp32, isOutput=False)
    bk_d = nc.declare_dram_parameter("bkr", [D, 1], fp32, isOutput=False)
    c_d = nc.declare_dram_parameter("cvec", [D, 1], fp32, isOutput=False)
    maskE_d = nc.declare_dram_parameter("maskE", [D, 1], fp32, isOutput=False)
    maskO_d = nc.declare_dram_parameter("maskO", [D, 1], fp32, isOutput=False)
    # output is produced transposed ([d, s] per t); host untransposes
    out_d = nc.declare_dram_parameter("out", [T, D, S], fp32, isOutput=True)

    with ExitStack() as ctx:
        tc = ctx.enter_context(tile.TileContext(nc))
        consts = ctx.enter_context(tc.tile_pool(name="consts", bufs=1))
        io = ctx.enter_context(tc.tile_pool(name="io", bufs=3))
        proj = ctx.enter_context(tc.tile_pool(name="proj", bufs=2))
        expp = ctx.enter_context(tc.tile_pool(name="expp", bufs=4))
        normp = ctx.enter_context(tc.tile_pool(name="normp", bufs=2))
        outp = ctx.enter_context(tc.tile_pool(name="outp", bufs=4))
        ps = ctx.enter_context(
            tc.tile_pool(name="ps", bufs=2, space=bass.MemorySpace.PSUM)
        )
        psav = ctx.enter_context(
            tc.tile_pool(name="psav", bufs=2, space=bass.MemorySpace.PSUM)
        )
        pspj = ctx.enter_context(
            tc.tile_pool(name="pspj", bufs=2, space=bass.MemorySpace.PSUM)
        )

        # ---- constants (loaded once) ----
        # weights go on the sync queue ahead of the t=0 input DMAs; the
        # later-needed consts ride the scalar/gpsimd queues so they don't
        # delay the first projection matmul.
        Wq_sb = consts.tile([D, D], bf16)
        Wk_sb = consts.tile([D, D], bf16)
        Wv_sb = consts.tile([D, D], bf16)
        Wo_sb = consts.tile([D, 3, D], bf16)
        bqe_sb = consts.tile([D, 1], fp32)
        bqo_sb = consts.tile([D, 1], fp32)
        bk_sb = consts.tile([D, 1], fp32)
        c_sb = consts.tile([D, 1], fp32)
        maskE_sb = consts.tile([D, 1], fp32)
        maskO_sb = consts.tile([D, 1], fp32)
        nc.sync.dma_start(out=Wq_sb, in_=Wq_d[:])
        nc.sync.dma_start(out=Wk_sb, in_=Wk_d[:])
        nc.sync.dma_start(out=Wv_sb, in_=Wv_d[:])
        nc.gpsimd.dma_start(out=Wo_sb, in_=Wo_d.rearrange("x p f -> p x f"))
        nc.scalar.dma_start(out=bqe_sb, in_=bqe_d[:])
        nc.scalar.dma_start(out=bqo_sb, in_=bqo_d[:])
        nc.scalar.dma_start(out=bk_sb, in_=bk_d[:])
        nc.scalar.dma_start(out=c_sb, in_=c_d[:])
        nc.scalar.dma_start(out=maskE_sb, in_=maskE_d[:])
        nc.scalar.dma_start(out=maskO_sb, in_=maskO_d[:])

        a_sb_hist = {}

        def emit_outproj(tt):
            # transposed output projection: outT[dout, s] = sum_d Wo[d, dout]
            # * attnT[d, s], accumulated per head block; +c via per-partition
            # scalar add fused into the PSUM->SBUF copy.
            # full-K matmuls per bank: the gap rows are zero on BOTH sides
            # (host zeros Wox's non-head rows; av_sb gaps stay memset-zero),
            # so K=128 contractions give exactly the per-head sum. Crucially
            # all matmuls of this accumulation group share tile_position
            # (0,0) -- accumulating at mixed bases faults the hardware.
            a_prev = a_sb_hist.pop(tt)
            ps_ot = pspj.tile([D, S], fp32, tag="pspj", name=f"ps_ot{tt}")
            for x in range(3):
                nc.tensor.matmul(
                    ps_ot,
                    Wo_sb[:, x, :],
                    a_prev[:, x, :],
                    start=(x == 0),
                    stop=(x == 2),
                )
            o_sb = outp.tile([D, S], fp32, tag="o_sb", name=f"o_sb{tt}")
            nc.vector.tensor_scalar_add(o_sb, ps_ot, c_sb)
            nc.sync.dma_start(out=out_d[tt % T], in_=o_sb)

        norm_steps = {}

        # manual double-buffer for av_sb (persistent tiles so the gap rows
        # between head blocks stay defined after a single startup memset;
        # Tile's access tracking handles the t/t+2 reuse hazards)
        av_sb_pair = []
        recipB_pair = []
        for i in range(2):
            av_sbP = normp.tile([D, 3, S], bf16, tag=f"av_sbP{i}", name=f"av_sbP{i}")
            nc.vector.memset(av_sbP, 0.0)
            av_sb_pair.append(av_sbP)
            # recipB gap + sumexp rows stay zero after one startup memset
            # (only the 16 per-head rows are rewritten), so a_sb = av * recipB
            # keeps zero gaps for the outproj contraction.
            rBP = normp.tile([D, 3, S], bf16, tag=f"recipBP{i}", name=f"recipBP{i}")
            nc.vector.memset(rBP, 0.0)
            recipB_pair.append(rBP)

        # gather row r=3j+bank of se9 holds head BANK_HEADS[bank][j]'s
        # sumexp (row 8 is the empty j=2/bankC slot; ln(0)=-inf there is
        # never consumed).
        SLOT = {0: 0, 3: 1, 6: 2, 1: 3, 4: 4, 7: 5, 2: 6, 5: 7}

        def make_norm_steps(t, av_sb):
            # The four pieces of softmax normalization for timestep t; they
            # are emitted interleaved into timestep t+1's head loop so the
            # PE/ACT streams never stall on this chain. All off the PE:
            # 1) gather the 8 sumexp rows into one [8, 512] SBUF tile via
            #    tiny SBUF->SBUF DMAs (partition-crossing moves)
            # 2) reciprocal on just those rows via exp(-ln(x)) on ACT
            # 3) broadcast reciprocals over each head's rows on GpSimd
            # 4) elementwise multiply -> normalized attn (a_sb); all-bf16
            #    SBUF operands make the DVE eligible for 2x mode
            state = {}

            def s0():
                # one partition-crossing DMA grabs all sumexp rows
                # (partitions 16/48/80 x 3 banks) in (j, bank) order
                se9 = normp.tile([9, S], bf16, tag="se9", name=f"se9{t}")
                nc.sync.dma_start(out=se9, in_=av_sb[16:81:32, :, :])
                state["se9"] = se9

            def s1():
                lnS = normp.tile([9, S], fp32, tag="lnS", name=f"lnS{t}")
                nc.scalar.activation(lnS, state["se9"], AF.Ln)
                recip_sb = normp.tile([9, S], bf16, tag="recip", name=f"recip{t}")
                nc.scalar.activation(recip_sb, lnS, AF.Exp, scale=-1.0)
                # gpsimd partition_broadcast needs partition-0 input; one
                # partition-crossing DMA flattens the rows onto partition 0.
                recipF = normp.tile([1, 9, S], bf16, tag="recipF", name=f"recipF{t}")
                nc.sync.dma_start(out=recipF, in_=recip_sb)
                state["recipF"] = recipF

            def s2():
                # gpsimd's partition_broadcast ucode only honors partition
                # base 0 on both APs, and the DVE big-multiply needs equal
                # input bases -- so broadcast each head's recip into a base-0
                # scratch tile, then DMA-scatter into the persistent recipB
                # at the head's 32j base.
                recipB = recipB_pair[t % 2]
                for h in range(H):
                    bank, j = head_bank(h)
                    rBh = normp.tile([HD, S], bf16, tag=f"rB{h}", name=f"rB{h}_{t}")
                    nc.gpsimd.partition_broadcast(
                        rBh, state["recipF"][0:1, SLOT[h], :]
                    )
                    nc.sync.dma_start(
                        out=recipB[32 * j : 32 * j + 16, bank, :], in_=rBh
                    )
                state["recipB"] = recipB

            def s3():
                a_sb = normp.tile([D, 3, S], bf16, tag="a_sb", name=f"a_sb{t}")
                nc.vector.tensor_mul(a_sb, av_sb, state["recipB"])
                a_sb_hist[t] = a_sb

            return [s0, s1, s2, s3]

        for rt in range(REPS * T):
            t = rt % T
            # ---- load inputs (already [D, S] transposed + bf16 on host) ----
            qT_in = io.tile([D, S], bf16, tag="qT_in")
            kT_in = io.tile([D, S], bf16, tag="kT_in")
            vT_in = io.tile([D, S], bf16, tag="vT_in")
            nc.sync.dma_start(out=qT_in, in_=qT_d[t])
            nc.sync.dma_start(out=kT_in, in_=kT_d[t])
            nc.sync.dma_start(out=vT_in, in_=vT_d[t])

            # ---- projections ----
            # masked parity copies (zero the other head of each 32-row pair)
            # with the bias fused: out = psum * mask + bias*mask.
            ps_qT = pspj.tile([D, S], fp32, tag="pspj")
            nc.tensor.matmul(ps_qT, Wq_sb, qT_in, start=True, stop=True)
            qT_ev = proj.tile([96, S], bf16, tag="qT_ev")
            qT_od = proj.tile([96, S], bf16, tag="qT_od")
            qT_ev3 = proj.tile([32, S], bf16, tag="qT_ev3")
            qT_od3 = proj.tile([32, S], bf16, tag="qT_od3")
            mlt, add = mybir.AluOpType.mult, mybir.AluOpType.add
            nc.vector.tensor_scalar(
                qT_ev, ps_qT[0:96, :], maskE_sb[0:96, :], bqe_sb[0:96, :], mlt, add
            )
            nc.vector.tensor_scalar(
                qT_od, ps_qT[0:96, :], maskO_sb[0:96, :], bqo_sb[0:96, :], mlt, add
            )
            nc.vector.tensor_scalar(
                qT_ev3, ps_qT[96:128, :], maskE_sb[96:128, :], bqe_sb[96:128, :],
                mlt, add,
            )
            nc.vector.tensor_scalar(
                qT_od3, ps_qT[96:128, :], maskO_sb[96:128, :], bqo_sb[96:128, :],
                mlt, add,
            )

            ps_kT = pspj.tile([D, S], fp32, tag="pspj")
            nc.tensor.matmul(ps_kT, Wk_sb, kT_in, start=True, stop=True)
            kT_sb = proj.tile([96, S], bf16, tag="kT_sb")
            kT_g3 = proj.tile([32, S], bf16, tag="kT_g3")
            nc.vector.tensor_scalar_add(kT_sb, ps_kT[0:96, :], bk_sb[0:96, :])
            nc.vector.tensor_scalar_add(kT_g3, ps_kT[96:128, :], bk_sb[96:128, :])

            # v in natural [s, d] layout, interleaved with ones columns:
            # v_sb[:, st, h, 0:16] = v[st*128+p, 16h+j], v_sb[:, st, h, 16] = 1
            ps_v = pspj.tile([D, S], fp32, tag="pspj")
            for st in range(NT):
                nc.tensor.matmul(
                    ps_v[:, st * 128 : (st + 1) * 128],
                    vT_in[:, st * 128 : (st + 1) * 128],
                    Wv_sb,
                    start=True,
                    stop=True,
                )
            v_sb = proj.tile([D, NT, H, HD + 1], bf16, tag="v_sb")
            nc.vector.memset(v_sb[:, :, :, HD : HD + 1], 1.0)
            nc.vector.tensor_copy(
                v_sb[:, :, :, 0:HD],
                ps_v.rearrange("p (st h j) -> p st h j", st=NT, h=H),
            )

            # ---- attention ----
            # per-head PSUM accumulator tile (bufs=2), copied into av_sb
            # right after each head completes; the previous timestep's
            # normalize steps are interleaved at head boundaries.
            av_sb = av_sb_pair[rt % 2]
            for h in range(H):
                g = h // 2
                if g < 3:
                    kT_l, qe, qo = kT_sb, qT_ev, qT_od
                    base = 32 * g
                else:
                    kT_l, qe, qo = kT_g3, qT_ev3, qT_od3
                    base = 0
                qT_par = qe if h % 2 == 0 else qo
                bank, j = head_bank(h)
                expT = expp.tile([D, NT, S], bf16, tag="expT")
                for stp in range(NT // 2):
                    ps_s = ps.tile([D, 2, S], fp32, tag="ps_s")
                    for k in range(2):
                        st = 2 * stp + k
                        nc.tensor.matmul(
                            ps_s[:, k, :],
                            kT_l[base : base + 32, st * 128 : (st + 1) * 128],
                            qT_par[base : base + 32, :],
                            start=True,
                            stop=True,
                        )
                    nc.scalar.activation(
                        expT[:, 2 * stp : 2 * stp + 2, :], ps_s, AF.Exp, scale=0.25
                    )
                av_h = psav.tile([17, S], fp32, tag="av_h")
                for st in range(NT):
                    nc.tensor.matmul(
                        av_h,
                        v_sb[:, st, h, :],
                        expT[:, st, :],
                        start=(st == 0),
                        stop=(st == NT - 1),
                    )
                nc.vector.tensor_copy(av_sb[32 * j : 32 * j + 17, bank, :], av_h)
                if rt > 0 and h < 4:
                    norm_steps[rt - 1][h]()

            # software-pipelined output projection for t-1: its inputs
            # (a_sb of t-1) finished during this t's head loop, so PE
            # never stalls on the normalize chain.
            if rt > 0:
                emit_outproj(rt - 1)

            norm_steps[rt] = make_norm_steps(rt, av_sb)

        for step in norm_steps[REPS * T - 1]:
            step()
        emit_outproj(REPS * T - 1)

    if PATCH_ACT_TABLES:
        bacc_mod.get_activation_tables = _patched_tables
    try:
        nc.compile()
    finally:
        bacc_mod.get_activation_tables = _orig_tables
    return nc


def make_in_maps(query, key, value, Wq, bq, Wk, bk, Wv, bv, Wo, bo):
    f = np.float32
    Wo = np.asarray(Wo, f)
    c = (np.asarray(bv, f) @ Wo + np.asarray(bo, f)).reshape(D, 1)
    Wox = np.zeros((3, D, D), f)
    for h in range(H):
        bank, j = head_bank(h)
        Wox[bank, 32 * j : 32 * j + 16, :] = Wo[HD * h : HD * (h + 1), :]
    maskE = np.zeros((D, 1), f)
    maskO = np.zeros((D, 1), f)
    for p in range(D):
        if (p % 32) < 16:
            maskE[p] = 1.0
        else:
            maskO[p] = 1.0
    bq = np.ascontiguousarray(bq, f).reshape(D, 1)
    shared = {
        "Wq": np.ascontiguousarray(Wq, f).astype(BF16),
        "Wk": np.ascontiguousarray(Wk, f).astype(BF16),
        "Wv": np.ascontiguousarray(Wv, f).astype(BF16),
        "Wox": Wox.astype(BF16),
        "bqe": bq * maskE,
        "bqo": bq * maskO,
        "bkr": np.ascontiguousarray(bk, f).reshape(D, 1),
        "cvec": c,
        "maskE": maskE,
        "maskO": maskO,
    }
    in_maps = []
    for b in range(B):
        m = dict(shared)
        m["qT"] = np.ascontiguousarray(
            np.asarray(query[b], f).transpose(0, 2, 1)
        ).astype(BF16)
        m["kT"] = np.ascontiguousarray(
            np.asarray(key[b], f).transpose(0, 2, 1)
        ).astype(BF16)
        m["vT"] = np.ascontiguousarray(
            np.asarray(value[b], f).transpose(0, 2, 1)
        ).astype(BF16)
        in_maps.append(m)
    return in_maps


def kernel(query, key, value, Wq, bq, Wk, bk, Wv, bv, Wo, bo):
    from concourse.bass_utils import run_bass_kernel_spmd

    nc = build_bass()
    in_maps = make_in_maps(query, key, value, Wq, bq, Wk, bk, Wv, bv, Wo, bo)
    res = run_bass_kernel_spmd(nc, in_maps, core_ids=list(range(B)))
    # device output is [T, D, S]; untranspose to [T, S, D]
    out = np.stack(
        [res.results[i]["out"].transpose(0, 2, 1) for i in range(B)]
    )
    return out


# revision 21
# speedup vs baseline: 1.1484x; 1.0146x over previous
"""Trainium2 Bass kernel for nn_AttentionLayer (B=8,T=12,S=512,D=128,H=8).

Sharding: data-parallel over batch; core b handles query/key/value[b].
Host pre-transposes per-(t) input slabs to [D, S] so every on-chip matmul
consumes operands with the contraction dim on partitions (no on-chip
transposes).

All matmul operand paths are bf16; PSUM accumulation stays fp32; the
softmax Ln stays fp32 (absolute error there is exponentiated).

Scheduling is built around the PE p-state: the PE only reaches full
clock (0.42 ns/col) when its instruction stream has no REAL stalls —
satisfied semaphore waits are free, actual idles drop it to half clock
for ~3us. So the stream is arranged as long pre-satisfied bursts:
  - scores for a head PAIR are ONE 1024-col matmul per s-tile: the
    moving operand is the two parity-masked q slabs side by side
    ([32, 2, S]), so both heads' scores land in one [D, 2, S] PSUM tile
    (parity, q) and exp covers both heads per instruction.
  - attnV runs one pair BEHIND scores, interleaved between score
    matmuls, so its exp dependencies are satisfied by emission time.
  - the softmax-normalization selector matmuls, outproj(t-1), and
    proj(t+1) fill the pair-boundary slots where the ACT deficit would
    otherwise stall the PE.

PE constraint: matmul operand/output APs may only start at partition
0/32/64 (lhsT and rhs at the SAME base). Hence the masked parity q
slabs at K=32 (pairs g=0,1,2 in a [0:96] slab, g3 at base 0), and:
  - attnV outputs go 3 heads per PSUM bank at offsets {0,32,64}
    (banks A: heads 0-2, B: 3-5, C: 6-7), each head writing
    rows [32j, 32j+17): 16 attn rows + 1 sumexp row (ones column in
    the stationary operand).
  - softmax denominator: av banks are copied to SBUF; a selector matmul
    per bank gathers the sumexp rows and broadcasts them over the head's
    16 rows; reciprocal = exp(-ln(x)) on ACT (Ln/Exp share a table set).
  - outproj accumulates per-bank K=128 matmuls (host zeros Wox gap
    rows, av_sb gaps stay memset-zero) plus a bias add with
    c = bv @ Wo + bo (bv folds through softmax: attn rows sum to 1).
"""

import sys

sys.path.insert(0, "/opt/trn_rl_repo")

import numpy as np
import ml_dtypes

BF16 = ml_dtypes.bfloat16

B, T, S, D = 8, 12, 512, 128
H, HD = 8, 16
NT = S // 128  # 4 s-tiles of 128
NG = 4  # head pairs
PATCH_ACT_TABLES = True

# head -> (bank, slot): banks A=0 (heads 0-2), B=1 (3-5), C=2 (6,7)
def head_bank(h):
    return (h // 3, h % 3) if h < 6 else (2, h - 6)


def build_bass():
    from contextlib import ExitStack

    import concourse.bass as bass
    from concourse import bacc, mybir
    import concourse.tile as tile

    fp32 = mybir.dt.float32
    bf16 = mybir.dt.bfloat16
    AF = mybir.ActivationFunctionType

    # Force Exp and Ln into the combined natural_log_exp_and_others table
    # set (one ACT table load for the whole kernel instead of ~1.3us
    # reloads twice per timestep).
    import concourse.hw_specs as hw_specs
    from concourse import bacc as bacc_mod

    _orig_tables = hw_specs.get_activation_tables

    def _patched_tables(arch):
        t = dict(_orig_tables(arch))
        for k in ("exp_and_others", "natural_log"):
            if k in t:
                t[k] = set()
        return t

    nc = bacc.Bacc(None, target_bir_lowering=False)

    qT_d = nc.declare_dram_parameter("qT", [T, D, S], bf16, isOutput=False)
    kT_d = nc.declare_dram_parameter("kT", [T, D, S], bf16, isOutput=False)
    vT_d = nc.declare_dram_parameter("vT", [T, D, S], bf16, isOutput=False)
    Wq_d = nc.declare_dram_parameter("Wq", [D, D], bf16, isOutput=False)
    Wk_d = nc.declare_dram_parameter("Wk", [D, D], bf16, isOutput=False)
    Wv_d = nc.declare_dram_parameter("Wv", [D, D], bf16, isOutput=False)
    Wo_d = nc.declare_dram_parameter("Wox", [3, D, D], bf16, isOutput=False)
    bqe_d = nc.declare_dram_parameter("bqe", [D, 1], fp32, isOutput=False)
    bqo_d = nc.declare_dram_parameter("bqo", [D, 1], fp32, isOutput=False)
    bk_d = nc.declare_dram_parameter("bkr", [D, 1], fp32, isOutput=False)
    c_d = nc.declare_dram_parameter("cvec", [D, 1], fp32, isOutput=False)
    selg_d = nc.declare_dram_parameter("selg", [3, 96, 8], bf16, isOutput=False)
    sel8_d = nc.declare_dram_parameter("sel8", [3, 8, D], bf16, isOutput=False)
    maskE_d = nc.declare_dram_parameter("maskE", [D, 1], fp32, isOutput=False)
    maskO_d = nc.declare_dram_parameter("maskO", [D, 1], fp32, isOutput=False)
    # output is produced transposed ([d, s] per t); host untransposes
    out_d = nc.declare_dram_parameter("out", [T, D, S], fp32, isOutput=True)

    with ExitStack() as ctx:
        tc = ctx.enter_context(tile.TileContext(nc))
        consts = ctx.enter_context(tc.tile_pool(name="consts", bufs=1))
        io = ctx.enter_context(tc.tile_pool(name="io", bufs=3))
        proj = ctx.enter_context(tc.tile_pool(name="proj", bufs=2))
        expp = ctx.enter_context(tc.tile_pool(name="expp", bufs=3))
        normp = ctx.enter_context(tc.tile_pool(name="normp", bufs=2))
        outp = ctx.enter_context(tc.tile_pool(name="outp", bufs=4))
        ps = ctx.enter_context(
            tc.tile_pool(name="ps", bufs=2, space=bass.MemorySpace.PSUM)
        )
        psav = ctx.enter_context(
            tc.tile_pool(name="psav", bufs=2, space=bass.MemorySpace.PSUM)
        )
        pspj = ctx.enter_context(
            tc.tile_pool(name="pspj", bufs=2, space=bass.MemorySpace.PSUM)
        )

        # ---- constants: weights on the sync queue ahead of t=0 inputs;
        # later-needed consts ride the scalar/gpsimd queues ----
        Wq_sb = consts.tile([D, D], bf16)
        Wk_sb = consts.tile([D, D], bf16)
        Wv_sb = consts.tile([D, D], bf16)
        Wo_sb = consts.tile([D, 3, D], bf16)
        bqe_sb = consts.tile([D, 1], fp32)
        bqo_sb = consts.tile([D, 1], fp32)
        bk_sb = consts.tile([D, 1], fp32)
        c_sb = consts.tile([D, 1], fp32)
        selg_sb = consts.tile([96, 3, 8], bf16)
        sel8_sb = consts.tile([8, 3, D], bf16)
        maskE_sb = consts.tile([D, 1], fp32)
        maskO_sb = consts.tile([D, 1], fp32)
        nc.sync.dma_start(out=Wq_sb, in_=Wq_d[:])
        nc.sync.dma_start(out=Wk_sb, in_=Wk_d[:])
        nc.sync.dma_start(out=Wv_sb, in_=Wv_d[:])
        nc.gpsimd.dma_start(out=Wo_sb, in_=Wo_d.rearrange("x p f -> p x f"))
        nc.gpsimd.dma_start(out=selg_sb, in_=selg_d.rearrange("x p f -> p x f"))
        nc.gpsimd.dma_start(out=sel8_sb, in_=sel8_d.rearrange("x p f -> p x f"))
        nc.scalar.dma_start(out=bqe_sb, in_=bqe_d[:])
        nc.scalar.dma_start(out=bqo_sb, in_=bqo_d[:])
        nc.scalar.dma_start(out=bk_sb, in_=bk_d[:])
        nc.scalar.dma_start(out=c_sb, in_=c_d[:])
        nc.scalar.dma_start(out=maskE_sb, in_=maskE_d[:])
        nc.scalar.dma_start(out=maskO_sb, in_=maskO_d[:])

        a_sb_hist = {}
        proj_tiles = {}

        mlt, add = mybir.AluOpType.mult, mybir.AluOpType.add

        def emit_input_dmas(t):
            qT_in = io.tile([D, S], bf16, tag="qT_in")
            kT_in = io.tile([D, S], bf16, tag="kT_in")
            vT_in = io.tile([D, S], bf16, tag="vT_in")
            nc.sync.dma_start(out=qT_in, in_=qT_d[t % T])
            nc.sync.dma_start(out=kT_in, in_=kT_d[t % T])
            nc.sync.dma_start(out=vT_in, in_=vT_d[t % T])
            proj_tiles[t] = {"in": (qT_in, kT_in, vT_in)}

        def emit_proj_q(t):
            qT_in = proj_tiles[t]["in"][0]
            ps_qT = pspj.tile([D, S], fp32, tag="pspj")
            nc.tensor.matmul(ps_qT, Wq_sb, qT_in, start=True, stop=True)
            # masked parity copies with the bias fused, interleaved so a
            # pair's two slabs sit side by side in the free dim:
            # qT_pair[:, 0, :] even-head-masked, [:, 1, :] odd-head-masked
            qT_pair = proj.tile([96, 2, S], bf16, tag="qT_pair")
            qT_pair3 = proj.tile([32, 2, S], bf16, tag="qT_pair3")
            nc.vector.tensor_scalar(
                qT_pair[:, 0, :], ps_qT[0:96, :], maskE_sb[0:96, :],
                bqe_sb[0:96, :], mlt, add,
            )
            nc.vector.tensor_scalar(
                qT_pair[:, 1, :], ps_qT[0:96, :], maskO_sb[0:96, :],
                bqo_sb[0:96, :], mlt, add,
            )
            nc.vector.tensor_scalar(
                qT_pair3[:, 0, :], ps_qT[96:128, :], maskE_sb[96:128, :],
                bqe_sb[96:128, :], mlt, add,
            )
            nc.vector.tensor_scalar(
                qT_pair3[:, 1, :], ps_qT[96:128, :], maskO_sb[96:128, :],
                bqo_sb[96:128, :], mlt, add,
            )
            proj_tiles[t]["q"] = (qT_pair, qT_pair3)

        def emit_proj_k(t):
            kT_in = proj_tiles[t]["in"][1]
            ps_kT = pspj.tile([D, S], fp32, tag="pspj")
            nc.tensor.matmul(ps_kT, Wk_sb, kT_in, start=True, stop=True)
            kT_sb = proj.tile([96, S], bf16, tag="kT_sb")
            kT_g3 = proj.tile([32, S], bf16, tag="kT_g3")
            nc.vector.tensor_scalar_add(kT_sb, ps_kT[0:96, :], bk_sb[0:96, :])
            nc.vector.tensor_scalar_add(kT_g3, ps_kT[96:128, :], bk_sb[96:128, :])
            proj_tiles[t]["k"] = (kT_sb, kT_g3)

        def emit_proj_v(t):
            vT_in = proj_tiles[t]["in"][2]
            # v in natural [s, d] layout, interleaved with ones columns
            ps_v = pspj.tile([D, S], fp32, tag="pspj")
            for st in range(NT):
                nc.tensor.matmul(
                    ps_v[:, st * 128 : (st + 1) * 128],
                    vT_in[:, st * 128 : (st + 1) * 128],
                    Wv_sb,
                    start=True,
                    stop=True,
                )
            v_sb = proj.tile([D, NT, H, HD + 1], bf16, tag="v_sb")
            nc.vector.memset(v_sb[:, :, :, HD : HD + 1], 1.0)
            nc.vector.tensor_copy(
                v_sb[:, :, :, 0:HD],
                ps_v.rearrange("p (st h j) -> p st h j", st=NT, h=H),
            )
            proj_tiles[t]["v"] = v_sb

        def emit_outproj(tt):
            a_prev = a_sb_hist.pop(tt)
            ps_ot = pspj.tile([D, S], fp32, tag="pspj", name=f"ps_ot{tt}")
            for x in range(3):
                nc.tensor.matmul(
                    ps_ot,
                    Wo_sb[:, x, :],
                    a_prev[:, x, :],
                    start=(x == 0),
                    stop=(x == 2),
                )
            o_sb = outp.tile([D, S], fp32, tag="o_sb", name=f"o_sb{tt}")
            nc.vector.tensor_scalar_add(o_sb, ps_ot, c_sb)
            nc.sync.dma_start(out=out_d[tt % T], in_=o_sb)

        # manual double-buffer for av_sb (persistent tiles so the gap rows
        # between head blocks stay defined after a single startup memset)
        av_sb_pair = []
        for i in range(2):
            av_sbP = normp.tile([D, 3, S], bf16, tag=f"av_sbP{i}", name=f"av_sbP{i}")
            nc.vector.memset(av_sbP, 0.0)
            av_sb_pair.append(av_sbP)

        norm_steps = {}

        def make_norm_steps(t, av_sb):
            # softmax normalization for timestep t (emitted during t+1):
            # 1) selector matmul gathers the 8 sumexp rows -> [8, S] psum
            # 2) reciprocal via exp(-ln(x)) on ACT
            # 3) selector matmuls broadcast recip over each head's rows
            # 4) elementwise multiply -> normalized attn (a_sb)
            state = {}

            def s0():
                ps_g = ps.tile([D, 2, S], fp32, tag="ps_s", name=f"ps_g{t}")
                for x in range(3):
                    nc.tensor.matmul(
                        ps_g[0:8, 0, :],
                        selg_sb[0:96, x, :],
                        av_sb[0:96, x, :],
                        start=(x == 0),
                        stop=(x == 2),
                    )
                state["ps_g"] = ps_g

            def s1():
                lnS = normp.tile([8, S], fp32, tag="lnS", name=f"lnS{t}")
                nc.scalar.activation(lnS, state["ps_g"][0:8, 0, :], AF.Ln)
                recip_sb = normp.tile([8, S], bf16, tag="recip", name=f"recip{t}")
                nc.scalar.activation(recip_sb, lnS, AF.Exp, scale=-1.0)
                state["recip"] = recip_sb

            def s2():
                ps_R1 = ps.tile([D, 2, S], fp32, tag="ps_s", name=f"ps_R1{t}")
                for x in range(2):
                    nc.tensor.matmul(
                        ps_R1[:, x, :],
                        sel8_sb[:, x, :],
                        state["recip"],
                        start=True,
                        stop=True,
                    )
                ps_R2 = ps.tile([D, 2, S], fp32, tag="ps_s", name=f"ps_R2{t}")
                nc.tensor.matmul(
                    ps_R2[:, 0, :],
                    sel8_sb[:, 2, :],
                    state["recip"],
                    start=True,
                    stop=True,
                )
                state["ps_R1"] = ps_R1
                state["ps_R2"] = ps_R2

            def s3():
                a_sb = normp.tile([D, 3, S], bf16, tag="a_sb", name=f"a_sb{t}")
                nc.vector.tensor_mul(
                    a_sb[:, 0:2, :], av_sb[:, 0:2, :], state["ps_R1"]
                )
                nc.vector.tensor_mul(
                    a_sb[:, 2, :], av_sb[:, 2, :], state["ps_R2"][:, 0, :]
                )
                a_sb_hist[t] = a_sb

            return [s0, s1, s2, s3]

        # ---- the pair-pipelined stream ----
        # state carried between pair slots
        pend_attnv = None  # (t, g) whose attnV is owed

        def emit_scores_pair(t, g, st_list):
            qT_pair, qT_pair3 = proj_tiles[t]["q"]
            kT_sb, kT_g3 = proj_tiles[t]["k"]
            if g < 3:
                kT_l, qp, base = kT_sb, qT_pair, 32 * g
            else:
                kT_l, qp, base = kT_g3, qT_pair3, 0
            ptiles = proj_tiles[t].setdefault("ps_sc", {})
            etile = proj_tiles[t].setdefault("expT", {})
            if g not in etile:
                etile[g] = expp.tile(
                    [D, NT, 2, S], bf16, tag="expT", name=f"expT_{t}_{g}"
                )
            for st in st_list:
                # one matmul per parity: a single matmul may not write
                # across PSUM banks, so the [D, 2, S] tile is filled
                # bank-by-bank; exp still covers both heads per instr.
                ps_s = ps.tile([D, 2, S], fp32, tag="ps_s")
                for par in range(2):
                    nc.tensor.matmul(
                        ps_s[:, par, :],
                        kT_l[base : base + 32, st * 128 : (st + 1) * 128],
                        qp[base : base + 32, par, :],
                        start=True,
                        stop=True,
                    )
                nc.scalar.activation(
                    etile[g][:, st, :, :], ps_s, AF.Exp, scale=0.25
                )
                ptiles[(g, st)] = ps_s

        def emit_attnv_head(t, g, par):
            h = 2 * g + par
            v_sb = proj_tiles[t]["v"]
            expT = proj_tiles[t]["expT"][g]
            bank, j = head_bank(h)
            av_h = psav.tile([17, S], fp32, tag="av_h")
            for st in range(NT):
                nc.tensor.matmul(
                    av_h,
                    v_sb[:, st, h, :],
                    expT[:, st, par, :],
                    start=(st == 0),
                    stop=(st == NT - 1),
                )
            av_sb = av_sb_pair[t % 2]
            nc.vector.tensor_copy(av_sb[32 * j : 32 * j + 17, bank, :], av_h)

        emit_input_dmas(0)
        emit_proj_q(0)
        emit_proj_k(0)
        emit_proj_v(0)

        for t in range(T):
            for g in range(NG):
                # scores st0, st1 of (t, g)
                emit_scores_pair(t, g, [0, 1])
                # attnV even head of the owed pair
                if pend_attnv is not None:
                    emit_attnv_head(*pend_attnv, 0)
                emit_scores_pair(t, g, [2])
                if pend_attnv is not None:
                    emit_attnv_head(*pend_attnv, 1)
                emit_scores_pair(t, g, [3])
                pend_attnv = (t, g)

                # pair-boundary fillers: keep the PE fed while ACT drains
                if g == 0:
                    if t + 1 < T:
                        emit_input_dmas(t + 1)  # prefetch, consumed at g3
                    if t >= 1:
                        norm_steps[t - 1][0]()  # s0 gather (PE)
                elif g == 1:
                    if t >= 1:
                        norm_steps[t - 1][1]()  # Ln/Exp (ACT)
                        norm_steps[t - 1][2]()  # s2 broadcast (PE)
                elif g == 2:
                    if t >= 1:
                        norm_steps[t - 1][3]()  # normalize mul (DVE)
                        emit_outproj(t - 1)
                else:
                    if t + 1 < T:
                        emit_proj_q(t + 1)
                        emit_proj_k(t + 1)
                        emit_proj_v(t + 1)

            norm_steps[t] = make_norm_steps(t, av_sb_pair[t % 2])

        # drain: attnV of the last pair, then the last norm + outproj
        emit_attnv_head(*pend_attnv, 0)
        emit_attnv_head(*pend_attnv, 1)
        for step in norm_steps[T - 1]:
            step()
        emit_outproj(T - 1)

    if PATCH_ACT_TABLES:
        bacc_mod.get_activation_tables = _patched_tables
    try:
        nc.compile()
    finally:
        bacc_mod.get_activation_tables = _orig_tables
    return nc


def make_in_maps(query, key, value, Wq, bq, Wk, bk, Wv, bv, Wo, bo):
    f = np.float32
    Wo = np.asarray(Wo, f)
    c = (np.asarray(bv, f) @ Wo + np.asarray(bo, f)).reshape(D, 1)
    selg = np.zeros((3, 96, 8), f)
    sel8 = np.zeros((3, 8, D), f)
    Wox = np.zeros((3, D, D), f)
    for h in range(H):
        bank, j = head_bank(h)
        selg[bank, 32 * j + 16, h] = 1.0
        sel8[bank, h, 32 * j : 32 * j + 32] = 1.0
        Wox[bank, 32 * j : 32 * j + 16, :] = Wo[HD * h : HD * (h + 1), :]
    maskE = np.zeros((D, 1), f)
    maskO = np.zeros((D, 1), f)
    for p in range(D):
        if (p % 32) < 16:
            maskE[p] = 1.0
        else:
            maskO[p] = 1.0
    bq = np.ascontiguousarray(bq, f).reshape(D, 1)
    shared = {
        "Wq": np.ascontiguousarray(Wq, f).astype(BF16),
        "Wk": np.ascontiguousarray(Wk, f).astype(BF16),
        "Wv": np.ascontiguousarray(Wv, f).astype(BF16),
        "Wox": Wox.astype(BF16),
        "bqe": bq * maskE,
        "bqo": bq * maskO,
        "bkr": np.ascontiguousarray(bk, f).reshape(D, 1),
        "cvec": c,
        "selg": selg.astype(BF16),
        "sel8": sel8.astype(BF16),
        "maskE": maskE,
        "maskO": maskO,
    }
    in_maps = []
    for b in range(B):
        m = dict(shared)
        m["qT"] = np.ascontiguousarray(
            np.asarray(query[b], f).transpose(0, 2, 1)
        ).astype(BF16)
        m["kT"] = np.ascontiguousarray(
            np.asarray(key[b], f).transpose(0, 2, 1)
        ).astype(BF16)
        m["vT"] = np.ascontiguousarray(
            np.asarray(value[b], f).transpose(0, 2, 1)
        ).astype(BF16)
        in_maps.append(m)
    return in_maps


def kernel(query, key, value, Wq, bq, Wk, bk, Wv, bv, Wo, bo):
    from concourse.bass_utils import run_bass_kernel_spmd

    nc = build_bass()
    in_maps = make_in_maps(query, key, value, Wq, bq, Wk, bk, Wv, bv, Wo, bo)
    res = run_bass_kernel_spmd(nc, in_maps, core_ids=list(range(B)))
    # device output is [T, D, S]; untranspose to [T, S, D]
    out = np.stack(
        [res.results[i]["out"].transpose(0, 2, 1) for i in range(B)]
    )
    return out


# revision 24
# speedup vs baseline: 1.1961x; 1.0416x over previous
"""Trainium2 Bass kernel for nn_AttentionLayer (B=8,T=12,S=512,D=128,H=8).

Sharding: data-parallel over batch; core b handles query/key/value[b].
Host pre-transposes per-(t) input slabs to [D, S] so every on-chip matmul
consumes operands with the contraction dim on partitions (no on-chip
transposes).

All matmul operand paths are bf16; PSUM accumulation stays fp32; the
softmax Ln stays fp32 (absolute error there is exponentiated).

Scheduling is built around the PE p-state: the PE only reaches full
clock (0.42 ns/col) when its instruction stream has no REAL stalls —
satisfied semaphore waits are free, actual idles drop it to half clock
for ~3us. So the stream is arranged as long pre-satisfied bursts:
  - scores for a head PAIR are ONE 1024-col matmul per s-tile: the
    moving operand is the two parity-masked q slabs side by side
    ([32, 2, S]), so both heads' scores land in one [D, 2, S] PSUM tile
    (parity, q) and exp covers both heads per instruction.
  - attnV runs one pair BEHIND scores, interleaved between score
    matmuls, so its exp dependencies are satisfied by emission time.
  - the softmax-normalization selector matmuls, outproj(t-1), and
    proj(t+1) fill the pair-boundary slots where the ACT deficit would
    otherwise stall the PE.

PE constraint: matmul operand/output APs may only start at partition
0/32/64 (lhsT and rhs at the SAME base). Hence the masked parity q
slabs at K=32 (pairs g=0,1,2 in a [0:96] slab, g3 at base 0), and:
  - attnV outputs go 3 heads per PSUM bank at offsets {0,32,64}
    (banks A: heads 0-2, B: 3-5, C: 6-7), each head writing
    rows [32j, 32j+17): 16 attn rows + 1 sumexp row (ones column in
    the stationary operand).
  - softmax denominator: av banks are copied to SBUF; a selector matmul
    per bank gathers the sumexp rows and broadcasts them over the head's
    16 rows; reciprocal = exp(-ln(x)) on ACT (Ln/Exp share a table set).
  - outproj accumulates per-bank K=128 matmuls (host zeros Wox gap
    rows, av_sb gaps stay memset-zero) plus a bias add with
    c = bv @ Wo + bo (bv folds through softmax: attn rows sum to 1).
"""

import sys

sys.path.insert(0, "/opt/trn_rl_repo")

import numpy as np
import ml_dtypes

BF16 = ml_dtypes.bfloat16

B, T, S, D = 8, 12, 512, 128
H, HD = 8, 16
NT = S // 128  # 4 s-tiles of 128
NG = 4  # head pairs
PATCH_ACT_TABLES = True

# head -> (bank, slot): banks A=0 (heads 0-2), B=1 (3-5), C=2 (6,7)
def head_bank(h):
    return (h // 3, h % 3) if h < 6 else (2, h - 6)


def build_bass():
    from contextlib import ExitStack

    import concourse.bass as bass
    from concourse import bacc, mybir
    import concourse.tile as tile

    fp32 = mybir.dt.float32
    bf16 = mybir.dt.bfloat16
    AF = mybir.ActivationFunctionType

    # Force Exp and Ln into the combined natural_log_exp_and_others table
    # set (one ACT table load for the whole kernel instead of ~1.3us
    # reloads twice per timestep).
    import concourse.hw_specs as hw_specs
    from concourse import bacc as bacc_mod

    _orig_tables = hw_specs.get_activation_tables

    def _patched_tables(arch):
        t = dict(_orig_tables(arch))
        for k in ("exp_and_others", "natural_log"):
            if k in t:
                t[k] = set()
        return t

    nc = bacc.Bacc(None, target_bir_lowering=False)

    qT_d = nc.declare_dram_parameter("qT", [T, D, S], bf16, isOutput=False)
    kT_d = nc.declare_dram_parameter("kT", [T, D, S], bf16, isOutput=False)
    vT_d = nc.declare_dram_parameter("vT", [T, D, S], bf16, isOutput=False)
    Wq_d = nc.declare_dram_parameter("Wq", [D, D], bf16, isOutput=False)
    Wk_d = nc.declare_dram_parameter("Wk", [D, D], bf16, isOutput=False)
    Wv_d = nc.declare_dram_parameter("Wv", [D, D], bf16, isOutput=False)
    Wo_d = nc.declare_dram_parameter("Wox", [3, D, D], bf16, isOutput=False)
    bqe_d = nc.declare_dram_parameter("bqe", [D, 1], fp32, isOutput=False)
    bqo_d = nc.declare_dram_parameter("bqo", [D, 1], fp32, isOutput=False)
    bk_d = nc.declare_dram_parameter("bkr", [D, 1], fp32, isOutput=False)
    c_d = nc.declare_dram_parameter("cvec", [D, 1], fp32, isOutput=False)
    selg_d = nc.declare_dram_parameter("selg", [3, 96, 8], bf16, isOutput=False)
    sel8_d = nc.declare_dram_parameter("sel8", [3, 8, D], bf16, isOutput=False)
    maskE_d = nc.declare_dram_parameter("maskE", [D, 1], fp32, isOutput=False)
    maskO_d = nc.declare_dram_parameter("maskO", [D, 1], fp32, isOutput=False)
    # output is produced transposed ([d, s] per t); host untransposes
    out_d = nc.declare_dram_parameter("out", [T, D, S], fp32, isOutput=True)

    with ExitStack() as ctx:
        tc = ctx.enter_context(tile.TileContext(nc))
        consts = ctx.enter_context(tc.tile_pool(name="consts", bufs=1))
        io = ctx.enter_context(tc.tile_pool(name="io", bufs=3))
        proj = ctx.enter_context(tc.tile_pool(name="proj", bufs=2))
        expp = ctx.enter_context(tc.tile_pool(name="expp", bufs=3))
        normp = ctx.enter_context(tc.tile_pool(name="normp", bufs=2))
        outp = ctx.enter_context(tc.tile_pool(name="outp", bufs=4))
        ps = ctx.enter_context(
            tc.tile_pool(name="ps", bufs=2, space=bass.MemorySpace.PSUM)
        )
        psav = ctx.enter_context(
            tc.tile_pool(name="psav", bufs=2, space=bass.MemorySpace.PSUM)
        )
        pspj = ctx.enter_context(
            tc.tile_pool(name="pspj", bufs=2, space=bass.MemorySpace.PSUM)
        )

        # ---- constants: weights on the sync queue ahead of t=0 inputs;
        # later-needed consts ride the scalar/gpsimd queues ----
        Wq_sb = consts.tile([D, D], bf16)
        Wk_sb = consts.tile([D, D], bf16)
        Wv_sb = consts.tile([D, D], bf16)
        Wo_sb = consts.tile([D, 3, D], bf16)
        bqe_sb = consts.tile([D, 1], fp32)
        bqo_sb = consts.tile([D, 1], fp32)
        bk_sb = consts.tile([D, 1], fp32)
        c_sb = consts.tile([D, 1], fp32)
        selg_sb = consts.tile([96, 3, 8], bf16)
        sel8_sb = consts.tile([8, 3, D], bf16)
        maskE_sb = consts.tile([D, 1], fp32)
        maskO_sb = consts.tile([D, 1], fp32)
        nc.sync.dma_start(out=Wq_sb, in_=Wq_d[:])
        nc.scalar.dma_start(out=Wk_sb, in_=Wk_d[:])
        nc.scalar.dma_start(out=Wv_sb, in_=Wv_d[:])
        nc.gpsimd.dma_start(out=Wo_sb, in_=Wo_d.rearrange("x p f -> p x f"))
        nc.gpsimd.dma_start(out=selg_sb, in_=selg_d.rearrange("x p f -> p x f"))
        nc.gpsimd.dma_start(out=sel8_sb, in_=sel8_d.rearrange("x p f -> p x f"))
        nc.scalar.dma_start(out=bqe_sb, in_=bqe_d[:])
        nc.scalar.dma_start(out=bqo_sb, in_=bqo_d[:])
        nc.scalar.dma_start(out=bk_sb, in_=bk_d[:])
        nc.scalar.dma_start(out=c_sb, in_=c_d[:])
        nc.scalar.dma_start(out=maskE_sb, in_=maskE_d[:])
        nc.scalar.dma_start(out=maskO_sb, in_=maskO_d[:])

        a_sb_hist = {}
        proj_tiles = {}

        mlt, add = mybir.AluOpType.mult, mybir.AluOpType.add

        def emit_input_dmas(t):
            qT_in = io.tile([D, S], bf16, tag="qT_in")
            kT_in = io.tile([D, S], bf16, tag="kT_in")
            vT_in = io.tile([D, S], bf16, tag="vT_in")
            nc.sync.dma_start(out=qT_in, in_=qT_d[t % T])
            nc.sync.dma_start(out=kT_in, in_=kT_d[t % T])
            nc.sync.dma_start(out=vT_in, in_=vT_d[t % T])
            proj_tiles[t] = {"in": (qT_in, kT_in, vT_in)}

        def emit_proj_q(t):
            qT_in = proj_tiles[t]["in"][0]
            ps_qT = pspj.tile([D, S], fp32, tag="pspj")
            nc.tensor.matmul(ps_qT, Wq_sb, qT_in, start=True, stop=True)
            # masked parity copies with the bias fused, interleaved so a
            # pair's two slabs sit side by side in the free dim:
            # qT_pair[:, 0, :] even-head-masked, [:, 1, :] odd-head-masked.
            # One DVE op per (pair, parity) so the first pair's slab is
            # ready as soon as possible for the next timestep's scores.
            qT_pair = proj.tile([96, 2, S], bf16, tag="qT_pair")
            qT_pair3 = proj.tile([32, 2, S], bf16, tag="qT_pair3")
            for g in range(3):
                r = slice(32 * g, 32 * g + 32)
                nc.vector.tensor_scalar(
                    qT_pair[r, 0, :], ps_qT[r, :], maskE_sb[r, :],
                    bqe_sb[r, :], mlt, add,
                )
                nc.vector.tensor_scalar(
                    qT_pair[r, 1, :], ps_qT[r, :], maskO_sb[r, :],
                    bqo_sb[r, :], mlt, add,
                )
            nc.vector.tensor_scalar(
                qT_pair3[:, 0, :], ps_qT[96:128, :], maskE_sb[96:128, :],
                bqe_sb[96:128, :], mlt, add,
            )
            nc.vector.tensor_scalar(
                qT_pair3[:, 1, :], ps_qT[96:128, :], maskO_sb[96:128, :],
                bqo_sb[96:128, :], mlt, add,
            )
            proj_tiles[t]["q"] = (qT_pair, qT_pair3)

        def emit_proj_k(t):
            kT_in = proj_tiles[t]["in"][1]
            ps_kT = pspj.tile([D, S], fp32, tag="pspj")
            nc.tensor.matmul(ps_kT, Wk_sb, kT_in, start=True, stop=True)
            kT_sb = proj.tile([96, S], bf16, tag="kT_sb")
            kT_g3 = proj.tile([32, S], bf16, tag="kT_g3")
            for g in range(3):
                r = slice(32 * g, 32 * g + 32)
                nc.vector.tensor_scalar_add(kT_sb[r, :], ps_kT[r, :], bk_sb[r, :])
            nc.vector.tensor_scalar_add(kT_g3, ps_kT[96:128, :], bk_sb[96:128, :])
            proj_tiles[t]["k"] = (kT_sb, kT_g3)

        def emit_proj_v(t):
            vT_in = proj_tiles[t]["in"][2]
            # v in natural [s, d] layout, interleaved with ones columns
            ps_v = pspj.tile([D, S], fp32, tag="pspj")
            for st in range(NT):
                nc.tensor.matmul(
                    ps_v[:, st * 128 : (st + 1) * 128],
                    vT_in[:, st * 128 : (st + 1) * 128],
                    Wv_sb,
                    start=True,
                    stop=True,
                )
            v_sb = proj.tile([D, NT, H, HD + 1], bf16, tag="v_sb")
            nc.vector.memset(v_sb[:, :, :, HD : HD + 1], 1.0)
            nc.vector.tensor_copy(
                v_sb[:, :, :, 0:HD],
                ps_v.rearrange("p (st h j) -> p st h j", st=NT, h=H),
            )
            proj_tiles[t]["v"] = v_sb

        def emit_outproj(tt):
            a_prev = a_sb_hist.pop(tt)
            ps_ot = pspj.tile([D, S], fp32, tag="pspj", name=f"ps_ot{tt}")
            for x in range(3):
                nc.tensor.matmul(
                    ps_ot,
                    Wo_sb[:, x, :],
                    a_prev[:, x, :],
                    start=(x == 0),
                    stop=(x == 2),
                )
            o_sb = outp.tile([D, S], fp32, tag="o_sb", name=f"o_sb{tt}")
            nc.vector.tensor_scalar_add(o_sb, ps_ot, c_sb)
            nc.sync.dma_start(out=out_d[tt % T], in_=o_sb)

        # manual double-buffer for av_sb (persistent tiles so the gap rows
        # between head blocks stay defined after a single startup memset)
        av_sb_pair = []
        for i in range(2):
            av_sbP = normp.tile([D, 3, S], bf16, tag=f"av_sbP{i}", name=f"av_sbP{i}")
            nc.vector.memset(av_sbP, 0.0)
            av_sb_pair.append(av_sbP)

        norm_steps = {}

        def make_norm_steps(t, av_sb):
            # softmax normalization for timestep t (emitted during t+1):
            # 1) selector matmul gathers the 8 sumexp rows -> [8, S] psum
            # 2) reciprocal via exp(-ln(x)) on ACT
            # 3) selector matmuls broadcast recip over each head's rows
            # 4) elementwise multiply -> normalized attn (a_sb)
            state = {}

            def s0():
                ps_g = ps.tile([D, 2, S], fp32, tag="ps_s", name=f"ps_g{t}")
                for x in range(3):
                    nc.tensor.matmul(
                        ps_g[0:8, 0, :],
                        selg_sb[0:96, x, :],
                        av_sb[0:96, x, :],
                        start=(x == 0),
                        stop=(x == 2),
                    )
                state["ps_g"] = ps_g

            def s1():
                lnS = normp.tile([8, S], fp32, tag="lnS", name=f"lnS{t}")
                nc.scalar.activation(lnS, state["ps_g"][0:8, 0, :], AF.Ln)
                recip_sb = normp.tile([8, S], bf16, tag="recip", name=f"recip{t}")
                nc.scalar.activation(recip_sb, lnS, AF.Exp, scale=-1.0)
                state["recip"] = recip_sb

            def s2():
                ps_R1 = ps.tile([D, 2, S], fp32, tag="ps_s", name=f"ps_R1{t}")
                for x in range(2):
                    nc.tensor.matmul(
                        ps_R1[:, x, :],
                        sel8_sb[:, x, :],
                        state["recip"],
                        start=True,
                        stop=True,
                    )
                ps_R2 = ps.tile([D, 2, S], fp32, tag="ps_s", name=f"ps_R2{t}")
                nc.tensor.matmul(
                    ps_R2[:, 0, :],
                    sel8_sb[:, 2, :],
                    state["recip"],
                    start=True,
                    stop=True,
                )
                state["ps_R1"] = ps_R1
                state["ps_R2"] = ps_R2

            def s3():
                a_sb = normp.tile([D, 3, S], bf16, tag="a_sb", name=f"a_sb{t}")
                nc.vector.tensor_mul(
                    a_sb[:, 0:2, :], av_sb[:, 0:2, :], state["ps_R1"]
                )
                nc.vector.tensor_mul(
                    a_sb[:, 2, :], av_sb[:, 2, :], state["ps_R2"][:, 0, :]
                )
                a_sb_hist[t] = a_sb

            return [s0, s1, s2, s3]

        # ---- the pair-pipelined stream ----
        # state carried between pair slots
        pend_attnv = None  # (t, g) whose attnV is owed

        def emit_scores_pair(t, g, st_list):
            qT_pair, qT_pair3 = proj_tiles[t]["q"]
            kT_sb, kT_g3 = proj_tiles[t]["k"]
            if g < 3:
                kT_l, qp, base = kT_sb, qT_pair, 32 * g
            else:
                kT_l, qp, base = kT_g3, qT_pair3, 0
            ptiles = proj_tiles[t].setdefault("ps_sc", {})
            etile = proj_tiles[t].setdefault("expT", {})
            if g not in etile:
                etile[g] = expp.tile(
                    [D, NT, 2, S], bf16, tag="expT", name=f"expT_{t}_{g}"
                )
            for st in st_list:
                # one matmul per parity: a single matmul may not write
                # across PSUM banks, so the [D, 2, S] tile is filled
                # bank-by-bank; exp still covers both heads per instr.
                ps_s = ps.tile([D, 2, S], fp32, tag="ps_s")
                for par in range(2):
                    nc.tensor.matmul(
                        ps_s[:, par, :],
                        kT_l[base : base + 32, st * 128 : (st + 1) * 128],
                        qp[base : base + 32, par, :],
                        start=True,
                        stop=True,
                    )
                nc.scalar.activation(
                    etile[g][:, st, :, :], ps_s, AF.Exp, scale=0.25
                )
                ptiles[(g, st)] = ps_s

        def emit_attnv_head(t, g, par):
            h = 2 * g + par
            v_sb = proj_tiles[t]["v"]
            expT = proj_tiles[t]["expT"][g]
            bank, j = head_bank(h)
            av_h = psav.tile([17, S], fp32, tag="av_h")
            for st in range(NT):
                nc.tensor.matmul(
                    av_h,
                    v_sb[:, st, h, :],
                    expT[:, st, par, :],
                    start=(st == 0),
                    stop=(st == NT - 1),
                )
            av_sb = av_sb_pair[t % 2]
            nc.vector.tensor_copy(av_sb[32 * j : 32 * j + 17, bank, :], av_h)

        emit_input_dmas(0)
        emit_proj_q(0)
        emit_proj_k(0)
        emit_proj_v(0)

        for t in range(T):
            for g in range(NG):
                # scores st0, st1 of (t, g)
                emit_scores_pair(t, g, [0, 1])
                # attnV even head of the owed pair
                if pend_attnv is not None:
                    emit_attnv_head(*pend_attnv, 0)
                emit_scores_pair(t, g, [2])
                if pend_attnv is not None:
                    emit_attnv_head(*pend_attnv, 1)
                emit_scores_pair(t, g, [3])
                pend_attnv = (t, g)

                # pair-boundary fillers: keep the PE fed while ACT drains,
                # and spread proj(t+1) so its DVE mask/bias ops finish well
                # before t+1's first score matmul needs them.
                if g == 0:
                    if t + 1 < T:
                        emit_input_dmas(t + 1)
                    if t >= 1:
                        norm_steps[t - 1][0]()  # s0 gather (PE)
                elif g == 1:
                    if t >= 1:
                        norm_steps[t - 1][1]()  # Ln/Exp (ACT)
                        norm_steps[t - 1][2]()  # s2 broadcast (PE)
                    if t + 1 < T:
                        emit_proj_q(t + 1)
                elif g == 2:
                    if t >= 1:
                        norm_steps[t - 1][3]()  # normalize mul (DVE)
                        emit_outproj(t - 1)
                    if t + 1 < T:
                        emit_proj_k(t + 1)
                else:
                    if t + 1 < T:
                        emit_proj_v(t + 1)

            norm_steps[t] = make_norm_steps(t, av_sb_pair[t % 2])

        # drain: attnV of the last pair, then the last norm + outproj
        emit_attnv_head(*pend_attnv, 0)
        emit_attnv_head(*pend_attnv, 1)
        for step in norm_steps[T - 1]:
            step()
        emit_outproj(T - 1)

    if PATCH_ACT_TABLES:
        bacc_mod.get_activation_tables = _patched_tables
    try:
        nc.compile()
    finally:
        bacc_mod.get_activation_tables = _orig_tables
    return nc


def make_in_maps(query, key, value, Wq, bq, Wk, bk, Wv, bv, Wo, bo):
    f = np.float32
    Wo = np.asarray(Wo, f)
    c = (np.asarray(bv, f) @ Wo + np.asarray(bo, f)).reshape(D, 1)
    selg = np.zeros((3, 96, 8), f)
    sel8 = np.zeros((3, 8, D), f)
    Wox = np.zeros((3, D, D), f)
    for h in range(H):
        bank, j = head_bank(h)
        selg[bank, 32 * j + 16, h] = 1.0
        sel8[bank, h, 32 * j : 32 * j + 32] = 1.0
        Wox[bank, 32 * j : 32 * j + 16, :] = Wo[HD * h : HD * (h + 1), :]
    maskE = np.zeros((D, 1), f)
    maskO = np.zeros((D, 1), f)
    for p in range(D):
        if (p % 32) < 16:
            maskE[p] = 1.0
        else:
            maskO[p] = 1.0
    bq = np.ascontiguousarray(bq, f).reshape(D, 1)
    shared = {
        "Wq": np.ascontiguousarray(Wq, f).astype(BF16),
        "Wk": np.ascontiguousarray(Wk, f).astype(BF16),
        "Wv": np.ascontiguousarray(Wv, f).astype(BF16),
        "Wox": Wox.astype(BF16),
        "bqe": bq * maskE,
        "bqo": bq * maskO,
        "bkr": np.ascontiguousarray(bk, f).reshape(D, 1),
        "cvec": c,
        "selg": selg.astype(BF16),
        "sel8": sel8.astype(BF16),
        "maskE": maskE,
        "maskO": maskO,
    }
    in_maps = []
    for b in range(B):
        m = dict(shared)
        m["qT"] = np.ascontiguousarray(
            np.asarray(query[b], f).transpose(0, 2, 1)
        ).astype(BF16)
        m["kT"] = np.ascontiguousarray(
            np.asarray(key[b], f).transpose(0, 2, 1)
        ).astype(BF16)
        m["vT"] = np.ascontiguousarray(
            np.asarray(value[b], f).transpose(0, 2, 1)
        ).astype(BF16)
        in_maps.append(m)
    return in_maps


def kernel(query, key, value, Wq, bq, Wk, bk, Wv, bv, Wo, bo):
    from concourse.bass_utils import run_bass_kernel_spmd

    nc = build_bass()
    in_maps = make_in_maps(query, key, value, Wq, bq, Wk, bk, Wv, bv, Wo, bo)
    res = run_bass_kernel_spmd(nc, in_maps, core_ids=list(range(B)))
    # device output is [T, D, S]; untranspose to [T, S, D]
    out = np.stack(
        [res.results[i]["out"].transpose(0, 2, 1) for i in range(B)]
    )
    return out
